# revision 31
# baseline (speedup 1.0000x reference)
"""Bass/Trainium2 kernel for nn_ALSHVGGNet (8 NeuronCores, data parallel).

Strategy:
- Batch 256 sharded 32/core; all conv/fc weights replicated (host-prepped fp16
  layouts); fp16 matmuls with f32 PSUM accumulation.
- BatchNorm uses full-batch statistics: per-layer per-channel (sum, sumsq)
  computed on-device via bn_stats and AllReduce'd across the 8 cores.
- Conv1/Conv2 (64 ch) run in a (parity, channel) packed layout: 2 images share
  the 128 partitions via block-diagonal weights so DVE/ACT epilogues use all
  128 lanes.
- Conv3 pairs taps along K (shifted activation copy on partitions 64..127).
- ALSH mask: filter codes precomputed on host from w6/hash_a (weights only);
  query code from the all-reduced act5 channel sums (sign-invariant
  simplification of the reference math); mask folded into BN6's affine.
- FC stack: act6 pooled activations AllGather'd, then every core computes the
  full-batch FC7/8/9 locally (BN7/8 stats become core-local).
"""

import os
import sys
import types

sys.path.insert(0, "/opt/trn_rl_repo")

import numpy as np

import concourse.bass as bass
import concourse.mybir as mybir
import concourse.tile as tile
from concourse import bacc
from concourse.bass_utils import run_bass_kernel_spmd

N_CORES = 8
SHARD = 32          # images per core
EPS = 1e-5
U = 0.999
F32 = mybir.dt.float32
FP16 = mybir.dt.float16
AX = mybir.AxisListType
ALU = mybir.AluOpType
AF = mybir.ActivationFunctionType

# global counts for BN stats normalization
N12 = 256 * 1024    # layers 1,2
N34 = 256 * 256     # layers 3,4
N56 = 256 * 64      # layers 5,6


def _install_ntff_hook():
    """Best effort registration of the axon NTFF profile hook (timing only)."""
    try:
        import antenv
        from trn_agent_boot.trn_boot import _ntff_profile_via_ctypes

        hooks = types.ModuleType("antenv.axon_hooks")
        hook = _ntff_profile_via_ctypes("/opt/axon/libaxon_pjrt.so")
        hooks.get_axon_ntff_profile_hook = lambda: hook
        hooks.set_axon_ntff_profile_hook = lambda h: None
        sys.modules["antenv.axon_hooks"] = hooks
        antenv.axon_hooks = hooks
    except Exception:
        pass


# ---------------------------------------------------------------------------
# Host-side input preparation
# ---------------------------------------------------------------------------

def _host_prep(inputs):
    """Build per-core and shared device input arrays from the raw inputs."""
    f16 = np.float16
    d = {}

    x = np.asarray(inputs["x"], np.float32)           # (256, 3, 32, 32)
    B = x.shape[0]
    assert B == N_CORES * SHARD

    # --- x im2col in (parity-block, tap, ci) x (pair, pix) layout ----------
    xp = np.zeros((B, 3, 34, 34), np.float32)
    xp[:, :, 1:33, 1:33] = x
    x1cols = []
    for core in range(N_CORES):
        sh = xp[core * SHARD:(core + 1) * SHARD]      # (32, 3, 34, 34)
        col = np.zeros((2, 9, 3, 16, 1024), np.float32)
        for dy in range(3):
            for dx in range(3):
                w = sh[:, :, dy:dy + 32, dx:dx + 32]  # (32, 3, 32, 32)
                w = w.reshape(16, 2, 3, 1024)
                col[:, dy * 3 + dx] = w.transpose(1, 2, 0, 3)
        x1cols.append(np.ascontiguousarray(
            col.reshape(54, 16 * 1024)).astype(f16))

    def w_tap(w):  # (O, I, 3, 3) -> [tap][I, O]
        return [np.ascontiguousarray(w[:, :, t // 3, t % 3].T) for t in range(9)]

    w1 = np.asarray(inputs["w1"], np.float32)
    w2 = np.asarray(inputs["w2"], np.float32)
    w3 = np.asarray(inputs["w3"], np.float32)
    w4 = np.asarray(inputs["w4"], np.float32)
    w5 = np.asarray(inputs["w5"], np.float32)
    w6 = np.asarray(inputs["w6"], np.float32)

    # L1 block-diag [54, 128]
    w1bd = np.zeros((54, 128), np.float32)
    for t in range(9):
        blk = w1[:, :, t // 3, t % 3].T               # (3 ci, 64 co)
        for par in range(2):
            w1bd[par * 27 + t * 3:par * 27 + t * 3 + 3, par * 64:par * 64 + 64] = blk
    d["w1bd"] = w1bd.astype(f16)

    # L2 block-diag per tap [128, 9, 128]
    w2t = w_tap(w2)
    w2bd = np.zeros((128, 9, 128), np.float32)
    for t in range(9):
        for par in range(2):
            w2bd[par * 64:par * 64 + 64, t, par * 64:par * 64 + 64] = w2t[t]
    d["w2bd"] = w2bd.astype(f16)

    # L3 tap-paired passes [128, 6, 128]: rows (s*64+ci)
    w3t = w_tap(w3)
    w3p = np.zeros((128, 6, 128), np.float32)
    for dy in range(3):
        w3p[0:64, 2 * dy, :] = w3t[dy * 3 + 0]
        w3p[64:128, 2 * dy, :] = w3t[dy * 3 + 1]
        w3p[0:64, 2 * dy + 1, :] = w3t[dy * 3 + 2]
    d["w3p"] = w3p.astype(f16)

    d["w4s"] = np.stack(w_tap(w4), axis=1).astype(f16)          # [128, 9, 128]
    d["w5s"] = np.stack(w_tap(w5), axis=1).astype(f16)          # [128, 9, 256]
    w6s = np.zeros((128, 9, 2, 256), np.float32)
    for t in range(9):
        wt = w6[:, :, t // 3, t % 3].T                           # (256 ci, 256 co)
        w6s[:, t, 0, :] = wt[0:128]
        w6s[:, t, 1, :] = wt[128:256]
    d["w6s"] = w6s.astype(f16)

    fc7 = np.asarray(inputs["fc7_w"], np.float32)                # (512, 4096)
    d["fc7s"] = np.ascontiguousarray(
        fc7.reshape(512, 2, 128, 16).transpose(2, 1, 3, 0)
        .reshape(128, 32, 512)).astype(f16)
    fc8 = np.asarray(inputs["fc8_w"], np.float32)                # (512, 512)
    d["fc8s"] = np.ascontiguousarray(
        fc8.T.reshape(4, 128, 512).transpose(1, 0, 2)).astype(f16)
    fc9 = np.asarray(inputs["fc9_w"], np.float32)                # (10, 512)
    d["fc9s"] = np.ascontiguousarray(
        fc9.T.reshape(4, 128, 10).transpose(1, 0, 2)).astype(f16)
    d["fc9bv"] = np.asarray(inputs["fc9_b"], np.float32).reshape(10, 1)

    # gamma/beta in stat layouts (f32)
    for i, shape in [(1, (64, 1)), (2, (64, 1)), (3, (128, 1)), (4, (128, 1))]:
        d[f"g{i}v"] = np.asarray(inputs[f"g{i}"], np.float32).reshape(shape)
        d[f"be{i}v"] = np.asarray(inputs[f"be{i}"], np.float32).reshape(shape)
    for i in (5, 6):
        d[f"g{i}v"] = np.ascontiguousarray(
            np.asarray(inputs[f"g{i}"], np.float32).reshape(2, 128).T)
        d[f"be{i}v"] = np.ascontiguousarray(
            np.asarray(inputs[f"be{i}"], np.float32).reshape(2, 128).T)
    for i in (7, 8):
        d[f"g{i}v"] = np.ascontiguousarray(
            np.asarray(inputs[f"g{i}"], np.float32).reshape(4, 128).T)
        d[f"be{i}v"] = np.ascontiguousarray(
            np.asarray(inputs[f"be{i}"], np.float32).reshape(4, 128).T)

    # fold/broadcast helpers for the (parity, channel) layers
    fold = np.zeros((128, 64), np.float32)
    bc = np.zeros((64, 128), np.float32)
    for c in range(64):
        fold[c, c] = fold[64 + c, c] = 1.0
        bc[c, c] = bc[c, 64 + c] = 1.0
    d["fold64"] = fold
    d["bc64"] = bc
    d["ones2"] = np.ones((2, 128), np.float32)
    d["id2"] = np.eye(2, dtype=np.float32)

    # --- ALSH host precompute (weights only) -------------------------------
    hash_a = np.asarray(inputs["hash_a"], np.float32)            # (2, 2306)
    wf = w6.reshape(256, -1)
    norms = np.linalg.norm(wf, axis=1)
    wf_s = wf * (U / norms.max())
    ns = np.linalg.norm(wf_s, axis=1)
    P = np.concatenate([wf_s, (ns ** 2)[:, None], (ns ** 4)[:, None]], axis=1)
    bits_f = (P @ hash_a.T) > 0                                   # (256, 2)
    f0 = bits_f[:, 0].astype(np.float32).reshape(2, 128).T        # [128, 2(mh)]
    f1 = bits_f[:, 1].astype(np.float32).reshape(2, 128).T
    d["f0b"] = np.ascontiguousarray(f0)
    d["f1b"] = np.ascontiguousarray(f1)
    ha9 = hash_a[:, :2304].reshape(2, 9, 256).sum(1)              # (2 bits, 256 c)
    ha9m = np.zeros((128, 2, 2), np.float32)                      # (p, mh, j)
    for mh in range(2):
        ha9m[:, mh, :] = ha9[:, mh * 128:(mh + 1) * 128].T
    d["ha9m"] = ha9m

    shared = d
    return x1cols, shared


SHARED_SPECS = {
    "w1bd": ((54, 128), FP16), "w2bd": ((128, 9, 128), FP16),
    "w3p": ((128, 6, 128), FP16), "w4s": ((128, 9, 128), FP16),
    "w5s": ((128, 9, 256), FP16), "w6s": ((128, 9, 2, 256), FP16),
    "fc7s": ((128, 32, 512), FP16), "fc8s": ((128, 4, 512), FP16),
    "fc9s": ((128, 4, 10), FP16), "fc9bv": ((10, 1), F32),
    "g1v": ((64, 1), F32), "be1v": ((64, 1), F32),
    "g2v": ((64, 1), F32), "be2v": ((64, 1), F32),
    "g3v": ((128, 1), F32), "be3v": ((128, 1), F32),
    "g4v": ((128, 1), F32), "be4v": ((128, 1), F32),
    "g5v": ((128, 2), F32), "be5v": ((128, 2), F32),
    "g6v": ((128, 2), F32), "be6v": ((128, 2), F32),
    "g7v": ((128, 4), F32), "be7v": ((128, 4), F32),
    "g8v": ((128, 4), F32), "be8v": ((128, 4), F32),
    "fold64": ((128, 64), F32), "bc64": ((64, 128), F32),
    "ones2": ((2, 128), F32), "id2": ((2, 2), F32),
    "f0b": ((128, 2), F32), "f1b": ((128, 2), F32),
    "ha9m": ((128, 2, 2), F32),
}

REPLICA = [list(range(N_CORES))]


def build_nc(debug_taps=()):
    nc = bacc.Bacc("TRN2", target_bir_lowering=False, debug=False,
                   num_devices=N_CORES)

    x1col_ext = nc.dram_tensor("x1col", [54, 16384], FP16, kind="ExternalInput")
    ext = {}
    for name, (shape, dt) in SHARED_SPECS.items():
        ext[name] = nc.dram_tensor(name, list(shape), dt, kind="ExternalInput")
    out_ext = nc.dram_tensor("out", [256, 10], F32, kind="ExternalOutput")
    dbg_ext = {}

    with tile.TileContext(nc) as tc:
        with (
            tc.tile_pool(name="const", bufs=1) as cpool,
            tc.tile_pool(name="acts", bufs=1) as apool,
            tc.tile_pool(name="scr", bufs=1) as spool,
            tc.tile_pool(name="psum", bufs=1, space="PSUM") as ppool,
            tc.tile_pool(name="dram", bufs=1, space="DRAM") as dpool,
        ):
            # ---- persistent consts/weights in SBUF (fc7s streamed later) ----
            sb = {}
            for name, (shape, dt) in SHARED_SPECS.items():
                if name == "fc7s":
                    continue
                t = cpool.tile(list(shape), dt, name=f"sb_{name}")
                nc.sync.dma_start(t[:], ext[name][:])
                sb[name] = t

            def dbg(name, ap):
                if name in debug_taps:
                    sh = [int(s) for s in ap.shape]
                    dt = ap.dtype
                    dbg_ext[name] = nc.dram_tensor(f"dbg_{name}", sh, dt,
                                                   kind="ExternalOutput")
                    nc.sync.dma_start(dbg_ext[name][:], ap)

            # ---- small helper chains ----
            def stat_combine(statv, T, half_cnt, name):
                """statv: [128, T, 6] bn_stats rows -> sums [128, 2] (sum, sumsq).

                half_cnt = per-tile even/odd element count (FD/2).
                """
                sm = spool.tile([128, 4], F32, name=f"sm_{name}")
                # sum of means (even + odd)
                nc.vector.tensor_reduce(sm[:, 0:1], statv[:, :, 1:2].squeeze(2),
                                        AX.X, ALU.add)
                nc.vector.tensor_reduce(sm[:, 1:2], statv[:, :, 4:5].squeeze(2),
                                        AX.X, ALU.add)
                # sum of count*var
                nc.vector.tensor_reduce(sm[:, 2:3], statv[:, :, 2:3].squeeze(2),
                                        AX.X, ALU.add)
                nc.vector.tensor_reduce(sm[:, 3:4], statv[:, :, 5:6].squeeze(2),
                                        AX.X, ALU.add)
                # sum of means^2
                msq = spool.tile([128, 2 * T], F32, name=f"msq_{name}")
                nc.vector.tensor_tensor(msq[:, 0:T], statv[:, :, 1:2].squeeze(2),
                                        statv[:, :, 1:2].squeeze(2), ALU.mult)
                nc.vector.tensor_tensor(msq[:, T:2 * T], statv[:, :, 4:5].squeeze(2),
                                        statv[:, :, 4:5].squeeze(2), ALU.mult)
                m2 = spool.tile([128, 1], F32, name=f"m2_{name}")
                nc.vector.tensor_reduce(m2[:], msq[:], AX.X, ALU.add)

                sums = spool.tile([128, 2], F32, name=f"sums_{name}")
                # sum = half_cnt * (sm0 + sm1)
                t0 = spool.tile([128, 1], F32, name=f"t0_{name}")
                nc.vector.tensor_tensor(t0[:], sm[:, 0:1], sm[:, 1:2], ALU.add)
                nc.vector.tensor_scalar_mul(sums[:, 0:1], t0[:], float(half_cnt))
                # sumsq = sm2 + sm3 + half_cnt * m2
                t1 = spool.tile([128, 1], F32, name=f"t1_{name}")
                nc.vector.tensor_tensor(t1[:], sm[:, 2:3], sm[:, 3:4], ALU.add)
                nc.vector.tensor_scalar(sums[:, 1:2], m2[:], float(half_cnt),
                                        None, ALU.mult)
                nc.vector.tensor_tensor(sums[:, 1:2], sums[:, 1:2], t1[:], ALU.add)
                return sums

            def allreduce(sums_ap, cols, name):
                ib = dpool.tile([128, cols], F32, name=f"arin_{name}")
                ob = dpool.tile([128, cols], F32, name=f"arout_{name}")
                nc.sync.dma_start(ib[:], sums_ap)
                nc.gpsimd.collective_compute(
                    "AllReduce", ALU.add, replica_groups=REPLICA,
                    ins=[ib.opt()], outs=[ob.opt()])
                g = spool.tile([128, cols], F32, name=f"gsum_{name}")
                nc.sync.dma_start(g[:], ob[:])
                return g

            def bn_affine_params(S, Q, g_ap, be_ap, n_total, P_, name,
                                 mask=None, k=1):
                """S,Q: [P_,k] global sum/sumsq -> (s, t) [P_,k] f32 tiles."""
                st = spool.tile([P_, 8 * k], F32, name=f"st_{name}")
                m, v, w, r0, a, dtmp, s_t, t_t = [st[:, i * k:(i + 1) * k]
                                                  for i in range(8)]
                nc.vector.tensor_scalar_mul(m, S, 1.0 / n_total)
                msq = spool.tile([P_, k], F32, name=f"stm_{name}")
                nc.vector.tensor_tensor(msq, m, m, ALU.mult)
                nc.vector.tensor_scalar_mul(v, Q, 1.0 / n_total)
                nc.vector.tensor_tensor(v, v, msq, ALU.subtract)
                if mask is not None:
                    nc.vector.tensor_tensor(m, m, mask, ALU.mult)
                    nc.vector.tensor_tensor(v, v, mask, ALU.mult)
                nc.vector.tensor_scalar_add(v, v, EPS)   # v := var + eps
                nc.vector.reciprocal(w, v)
                nc.scalar.activation(r0, w, AF.Sqrt)
                # Newton step: r1 = r0 * (1.5 - 0.5 * v * r0^2)
                nc.vector.tensor_tensor(a, r0, r0, ALU.mult)
                nc.vector.tensor_tensor(a, a, v, ALU.mult)
                nc.vector.tensor_scalar(a, a, -0.5, 1.5, ALU.mult, ALU.add)
                nc.vector.tensor_tensor(r0, r0, a, ALU.mult)
                nc.vector.tensor_tensor(s_t, g_ap, r0, ALU.mult)
                if mask is not None:
                    nc.vector.tensor_tensor(s_t, s_t, mask, ALU.mult)
                nc.vector.tensor_tensor(dtmp, m, s_t, ALU.mult)
                nc.vector.tensor_tensor(t_t, be_ap, dtmp, ALU.subtract)
                return s_t, t_t

            def fold_bcast(sums, g_ap, be_ap, n_total, name):
                """(parity, channel) stats: fold to 64, bn math, broadcast to 128."""
                up = spool.tile([64, 2], F32, name=f"up_{name}")
                nc.vector.tensor_copy(up[:], sums[64:128, :])
                s64 = spool.tile([64, 2], F32, name=f"s64_{name}")
                nc.vector.tensor_tensor(s64[:], sums[0:64, :], up[:], ALU.add)
                s_t, t_t = bn_affine_params(s64[:, 0:1], s64[:, 1:2],
                                            g_ap, be_ap, n_total, 64, name)
                st128 = spool.tile([128, 2], F32, name=f"stb_{name}")
                nc.vector.tensor_copy(st128[0:64, 0:1], s_t)
                nc.vector.tensor_copy(st128[0:64, 1:2], t_t)
                nc.vector.tensor_copy(st128[64:128, :], st128[0:64, :])
                return st128[:, 0:1], st128[:, 1:2]

            # Barrier/warmup collective: aligns core start times so the
            # first real AllReduce doesn't absorb startup skew.
            wib = dpool.tile([128, 1], F32, name="warm_ib")
            wob = dpool.tile([128, 1], F32, name="warm_ob")
            nc.gpsimd.collective_compute(
                "AllReduce", ALU.add, replica_groups=REPLICA,
                ins=[wib.opt()], outs=[wob.opt()])

            # ================= Layer 1 =================
            y1 = apool.tile([128, 16384], FP16, name="y1", tag="ybuf")
            stat1 = spool.tile([128, 32, 6], F32, name="stat1")
            for t in range(32):
                if t % 8 == 0:
                    x1t = spool.tile([54, 8, 512], FP16, name="x1t", tag="x1t",
                                     bufs=3)
                    nc.gpsimd.dma_start(
                        x1t[:], x1col_ext[:, t * 512:(t + 8) * 512]
                        .rearrange("p (a b) -> p a b", a=8))
                ps = ppool.tile([128, 512], F32, name=f"ps1_{t}", tag="ps", bufs=8)
                nc.tensor.matmul(ps[:], sb["w1bd"][:], x1t[:, t % 8, :],
                                 start=True, stop=True)
                nc.vector.bn_stats(stat1[:, t, :], ps[:])
                nc.scalar.copy(y1[:, t * 512:(t + 1) * 512], ps[:])
            dbg("y1", y1[:])
            sums1 = stat_combine(stat1, 32, 256, "L1")
            gs1 = allreduce(sums1[:], 2, "L1")
            s1, t1 = fold_bcast(gs1, sb["g1v"][:], sb["be1v"][:], N12, "L1")

            act1 = apool.tile([128, 16, 34, 34], FP16, name="act1", tag="actpad")
            nc.gpsimd.memset(act1[:, :, 0:1, :], 0.0)
            nc.gpsimd.memset(act1[:, :, 33:34, :], 0.0)
            nc.gpsimd.memset(act1[:, :, 1:33, 0:1], 0.0)
            nc.gpsimd.memset(act1[:, :, 1:33, 33:34], 0.0)
            y1v = y1.rearrange("p (pr y x) -> p pr y x", pr=16, y=32, x=32)
            for p0, pn in [(0, 1), (1, 3), (4, 4), (8, 4), (12, 4)]:
                nc.scalar.activation(act1[:, p0:p0 + pn, 1:33, 1:33],
                                     y1v[:, p0:p0 + pn], AF.Relu,
                                     bias=t1, scale=s1)
            dbg("act1", act1[:])

            # ================= Layer 2 =================
            y2 = apool.tile([128, 16384], FP16, name="y2", tag="ybuf")
            stat2 = spool.tile([128, 32, 6], F32, name="stat2")
            for pr in range(16):
                pss = [ppool.tile([128, 512], F32, name=f"ps2_{pr}_{h}",
                                  tag="ps", bufs=8) for h in range(2)]
                for t in range(9):
                    dy, dx = t // 3, t % 3
                    for h in range(2):
                        rhs = act1[:, pr, h * 16 + dy:h * 16 + dy + 16,
                                   dx:dx + 32]
                        nc.tensor.matmul(pss[h][:], sb["w2bd"][:, t, :], rhs,
                                         start=(t == 0), stop=(t == 8))
                for h in range(2):
                    ti = pr * 2 + h
                    nc.vector.bn_stats(stat2[:, ti, :], pss[h][:])
                    nc.scalar.copy(y2[:, ti * 512:(ti + 1) * 512], pss[h][:])
            dbg("y2", y2[:])
            sums2 = stat_combine(stat2, 32, 256, "L2")
            gs2 = allreduce(sums2[:], 2, "L2")
            s2, t2 = fold_bcast(gs2, sb["g2v"][:], sb["be2v"][:], N12, "L2")

            act2f = apool.tile([128, 16, 1024], FP16, name="act2f", tag="actfull")
            y2v = y2.rearrange("p (pr q) -> p pr q", pr=16)
            a2v = act2f.rearrange("p pr (y x two) -> p pr y x two", y=32, two=2)
            pl1 = apool.tile([128, 16, 32, 16], FP16, name="pl1", tag="scr16")
            p1v = pl1.rearrange("p pr (y two) x -> p pr y two x", two=2)
            cpar = apool.tile([128, 16, 256], FP16, name="cpar", tag="cparscr")
            cpv = cpar.rearrange("p pr (y x) -> p pr y x", y=16)

            # scatter into act2p [128=(s,c), 32 img, 18, 18] with shift copy
            act2p = apool.tile([128, 32, 18, 18], FP16, name="act2p", tag="actp")
            nc.gpsimd.memset(act2p[:, :, 0:1, :], 0.0)
            nc.gpsimd.memset(act2p[:, :, 17:18, :], 0.0)
            nc.gpsimd.memset(act2p[0:64, :, 1:17, 0:1], 0.0)
            nc.gpsimd.memset(act2p[0:64, :, 1:17, 17:18], 0.0)
            nc.gpsimd.memset(act2p[64:128, :, 1:17, 16:18], 0.0)
            a2pv = act2p.rearrange("p (i ip) y x -> p i ip y x", ip=2)
            cp4 = cpar.rearrange("p pr (y x) -> p pr y x", y=16)
            for p0 in range(0, 16, 4):
                sl = slice(p0, p0 + 4)
                nc.scalar.activation(act2f[:, sl, :], y2v[:, sl],
                                     AF.Relu, bias=t2, scale=s2)
                nc.vector.tensor_tensor(pl1[:, sl],
                                        a2v[:, sl, :, :, 0:1].squeeze(4),
                                        a2v[:, sl, :, :, 1:2].squeeze(4),
                                        ALU.max)
                nc.vector.tensor_tensor(cpv[:, sl],
                                        p1v[:, sl, :, 0:1, :].squeeze(3),
                                        p1v[:, sl, :, 1:2, :].squeeze(3),
                                        ALU.max)
                nc.vector.tensor_copy(a2pv[0:64, sl, 0, 1:17, 1:17],
                                      cp4[0:64, sl])
                nc.gpsimd.tensor_copy(a2pv[64:128, sl, 1, 1:17, 0:16],
                                      cp4[64:128, sl])
                for pr in range(p0, p0 + 4):
                    nc.gpsimd.dma_start(a2pv[0:64, pr, 1, 1:17, 1:17],
                                        cp4[64:128, pr])
                    nc.gpsimd.dma_start(a2pv[64:128, pr, 0, 1:17, 0:16],
                                        cp4[0:64, pr])
            dbg("act2p", act2p[:])

            # ================= Layer 3 =================
            y3 = apool.tile([128, 32, 256], FP16, name="y3", tag="ybuf")
            stat3 = spool.tile([128, 16, 6], F32, name="stat3")
            passes = [(0, 0), (0, 2), (1, 0), (1, 2), (2, 0), (2, 2)]
            for tg in range(8):
                pss = [ppool.tile([128, 512], F32, name=f"ps3_{tg}_{u}",
                                  tag="ps", bufs=8) for u in range(2)]
                for pi, (dy, dx) in enumerate(passes):
                    for u in range(2):
                        i0 = (tg * 2 + u) * 2
                        rhs = act2p[:, i0:i0 + 2, dy:dy + 16, dx:dx + 16]
                        nc.tensor.matmul(pss[u][:],
                                         sb["w3p"][:, 2 * dy + (dx // 2), :],
                                         rhs, start=(pi == 0), stop=(pi == 5))
                for u in range(2):
                    ti = tg * 2 + u
                    i0 = ti * 2
                    nc.vector.bn_stats(stat3[:, ti, :], pss[u][:])
                    nc.scalar.copy(
                        y3[:, i0:i0 + 2, :].rearrange("p a b -> p (a b)"),
                        pss[u][:])
            dbg("y3", y3[:])
            sums3 = stat_combine(stat3, 16, 256, "L3")
            gs3 = allreduce(sums3[:], 2, "L3")
            s3, t3 = bn_affine_params(gs3[:, 0:1], gs3[:, 1:2], sb["g3v"][:],
                                      sb["be3v"][:], N34, 128, "L3")

            act3 = apool.tile([128, 32, 18, 18], FP16, name="act3", tag="actpad")
            nc.gpsimd.memset(act3[:, :, 0:1, :], 0.0)
            nc.gpsimd.memset(act3[:, :, 17:18, :], 0.0)
            nc.gpsimd.memset(act3[:, :, 1:17, 0:1], 0.0)
            nc.gpsimd.memset(act3[:, :, 1:17, 17:18], 0.0)
            y3v = y3.rearrange("p i (y x) -> p i y x", y=16)
            for i0, ni in [(0, 2), (2, 6), (8, 8), (16, 8), (24, 8)]:
                nc.scalar.activation(act3[:, i0:i0 + ni, 1:17, 1:17],
                                     y3v[:, i0:i0 + ni], AF.Relu,
                                     bias=t3, scale=s3)
            dbg("act3", act3[:])

            # ================= Layer 4 =================
            y4 = apool.tile([128, 32, 256], FP16, name="y4", tag="ybuf")
            stat4 = spool.tile([128, 16, 6], F32, name="stat4")
            for tg in range(8):
                pss = [ppool.tile([128, 512], F32, name=f"ps4_{tg}_{u}",
                                  tag="ps", bufs=8) for u in range(2)]
                for t in range(9):
                    dy, dx = t // 3, t % 3
                    for u in range(2):
                        i0 = (tg * 2 + u) * 2
                        rhs = act3[:, i0:i0 + 2, dy:dy + 16, dx:dx + 16]
                        nc.tensor.matmul(pss[u][:], sb["w4s"][:, t, :], rhs,
                                         start=(t == 0), stop=(t == 8))
                for u in range(2):
                    ti = tg * 2 + u
                    i0 = ti * 2
                    nc.vector.bn_stats(stat4[:, ti, :], pss[u][:])
                    nc.scalar.copy(
                        y4[:, i0:i0 + 2, :].rearrange("p a b -> p (a b)"),
                        pss[u][:])
            dbg("y4", y4[:])
            sums4 = stat_combine(stat4, 16, 256, "L4")
            gs4 = allreduce(sums4[:], 2, "L4")
            s4, t4 = bn_affine_params(gs4[:, 0:1], gs4[:, 1:2], sb["g4v"][:],
                                      sb["be4v"][:], N34, 128, "L4")

            act4f = apool.tile([128, 32, 256], FP16, name="act4f", tag="actfull")
            y4v = y4.rearrange("p i (y x) -> p i y x", y=16)
            a4fv = act4f.rearrange("p i (y x) -> p i y x", y=16)
            a4v = act4f.rearrange("p i (y x two) -> p i y x two", y=16, two=2)
            pl2 = apool.tile([128, 32, 16, 8], FP16, name="pl2", tag="scr16")
            p2v = pl2.rearrange("p i (y two) x -> p i y two x", two=2)
            for i0 in range(0, 32, 8):
                sl = slice(i0, i0 + 8)
                nc.scalar.activation(a4fv[:, sl], y4v[:, sl],
                                     AF.Relu, bias=t4, scale=s4)
                nc.vector.tensor_tensor(pl2[:, sl],
                                        a4v[:, sl, :, :, 0:1].squeeze(4),
                                        a4v[:, sl, :, :, 1:2].squeeze(4),
                                        ALU.max)
            act4p = apool.tile([128, 32, 10, 10], FP16, name="act4p", tag="actp")
            nc.gpsimd.memset(act4p[:, :, 0:1, :], 0.0)
            nc.gpsimd.memset(act4p[:, :, 9:10, :], 0.0)
            nc.gpsimd.memset(act4p[:, :, 1:9, 0:1], 0.0)
            nc.gpsimd.memset(act4p[:, :, 1:9, 9:10], 0.0)
            for i0 in range(0, 32, 8):
                sl = slice(i0, i0 + 8)
                nc.vector.tensor_tensor(act4p[:, sl, 1:9, 1:9],
                                        p2v[:, sl, :, 0:1, :].squeeze(3),
                                        p2v[:, sl, :, 1:2, :].squeeze(3),
                                        ALU.max)
            dbg("act4p", act4p[:])

            # ================= Layer 5 =================
            y5 = apool.tile([128, 2, 32, 64], FP16, name="y5", tag="ybuf")
            stat5 = spool.tile([128, 2, 4, 6], F32, name="stat5")
            for mh in range(2):
                for tg in range(2):
                    pss = [ppool.tile([128, 512], F32, name=f"ps5_{mh}_{tg}_{u}",
                                      tag="ps", bufs=8) for u in range(2)]
                    for t in range(9):
                        dy, dx = t // 3, t % 3
                        for u in range(2):
                            i0 = (tg * 2 + u) * 8
                            rhs = act4p[:, i0:i0 + 8, dy:dy + 8, dx:dx + 8]
                            nc.tensor.matmul(
                                pss[u][:], sb["w5s"][:, t, mh * 128:(mh + 1) * 128],
                                rhs, start=(t == 0), stop=(t == 8))
                    for u in range(2):
                        ti = tg * 2 + u
                        i0 = ti * 8
                        nc.vector.bn_stats(stat5[:, mh, ti, :], pss[u][:])
                        nc.scalar.copy(
                            y5[:, mh, i0:i0 + 8, :].rearrange("p a b -> p (a b)"),
                            pss[u][:])
            dbg("y5", y5[:])
            sums5 = spool.tile([128, 4], F32, name="sums5")
            for h in range(2):
                sums5h = stat_combine(stat5[:, h], 4, 256, f"L5h{h}")
                nc.vector.tensor_copy(sums5[:, 2 * h:2 * h + 2], sums5h[:])
            gs5 = allreduce(sums5[:], 4, "L5")

            act5 = apool.tile([128, 2, 32, 10, 10], FP16, name="act5",
                              tag="actpad")
            for h in range(2):
                nc.gpsimd.memset(act5[:, h, :, 0:1, :], 0.0)
                nc.gpsimd.memset(act5[:, h, :, 9:10, :], 0.0)
                nc.gpsimd.memset(act5[:, h, :, 1:9, 0:1], 0.0)
                nc.gpsimd.memset(act5[:, h, :, 1:9, 9:10], 0.0)
            qsum = spool.tile([128, 2], F32, name="qsum")
            y5v = y5.rearrange("p mh i (y x) -> p mh i y x", y=8)
            for h in range(2):
                s5, t5 = bn_affine_params(gs5[:, 2 * h:2 * h + 1],
                                          gs5[:, 2 * h + 1:2 * h + 2],
                                          sb["g5v"][:, h:h + 1],
                                          sb["be5v"][:, h:h + 1],
                                          N56, 128, f"L5h{h}")
                nc.scalar.activation(act5[:, h, :, 1:9, 1:9], y5v[:, h],
                                     AF.Relu, bias=t5, scale=s5,
                                     accum_out=qsum[:, h:h + 1])
            dbg("act5", act5[:])

            # ================= Layer 6 =================
            # All 8 psum tiles open; K-chunk h=0 matmuls run for every tile
            # first, so the h=1 chunk (which needs act5 h1 / AR5b) comes last.
            y6 = apool.tile([128, 2, 32, 64], FP16, name="y6", tag="scr16")
            stat6 = spool.tile([128, 2, 4, 6], F32, name="stat6")
            ps6 = {}
            for mh in range(2):
                for ti in range(4):
                    ps6[(mh, ti)] = ppool.tile([128, 512], F32,
                                               name=f"ps6_{mh}_{ti}",
                                               tag="ps", bufs=8)
            for h in range(2):
                for t in range(9):
                    dy, dx = t // 3, t % 3
                    for mh in range(2):
                        for ti in range(4):
                            i0 = ti * 8
                            rhs = act5[:, h, i0:i0 + 8, dy:dy + 8, dx:dx + 8]
                            nc.tensor.matmul(
                                ps6[(mh, ti)][:],
                                sb["w6s"][:, t, h, mh * 128:(mh + 1) * 128],
                                rhs, start=(h == 0 and t == 0),
                                stop=(h == 1 and t == 8))
            for mh in range(2):
                for ti in range(4):
                    i0 = ti * 8
                    nc.vector.bn_stats(stat6[:, mh, ti, :], ps6[(mh, ti)][:])
                    nc.scalar.copy(
                        y6[:, mh, i0:i0 + 8, :].rearrange("p a b -> p (a b)"),
                        ps6[(mh, ti)][:])
            dbg("y6", y6[:])
            sums6 = spool.tile([128, 6], F32, name="sums6")
            for h in range(2):
                sums6h = stat_combine(stat6[:, h], 4, 256, f"L6h{h}")
                nc.vector.tensor_copy(sums6[:, 2 * h:2 * h + 2], sums6h[:])
            nc.vector.tensor_copy(sums6[:, 4:6], qsum[:])
            gs6 = allreduce(sums6[:], 6, "L6")

            # ---- ALSH mask from global qsums ----
            psd = ppool.tile([2, 2], F32, name="psd", tag="ps", bufs=8)
            for mh in range(2):
                nc.tensor.matmul(psd[:, 0:1], sb["ha9m"][:, mh, :],
                                 gs6[:, 4 + mh:5 + mh],
                                 start=(mh == 0), stop=(mh == 1))
            dsb = spool.tile([2, 2], F32, name="dsb")
            nc.vector.tensor_copy(dsb[:, 0:1], psd[:, 0:1])
            bq = spool.tile([2, 2], F32, name="bq")
            nc.vector.tensor_scalar(bq[:, 0:1], dsb[:, 0:1], 0.0, None, ALU.is_gt)
            bqd = spool.tile([2, 2], F32, name="bqd")
            nc.vector.tensor_scalar_mul(bqd[:], sb["id2"][:], bq[:, 0:1])
            psb2 = ppool.tile([128, 2], F32, name="psb2", tag="ps", bufs=8)
            nc.tensor.matmul(psb2[:], sb["ones2"][:], bqd[:],
                             start=True, stop=True)
            bqb = spool.tile([128, 2], F32, name="bqb")
            nc.vector.tensor_copy(bqb[:], psb2[:])
            mask = spool.tile([128, 2], F32, name="mask")
            e0 = spool.tile([128, 2], F32, name="e0")
            e0s = spool.tile([128, 2], F32, name="e0s")
            nc.vector.tensor_scalar(e0[:], sb["f0b"][:], bqb[:, 0:1], None,
                                    ALU.subtract)
            nc.scalar.activation(e0s[:], e0[:], AF.Square)
            nc.vector.tensor_scalar(mask[:], e0s[:], -1.0, 1.0, ALU.mult, ALU.add)
            e1 = spool.tile([128, 2], F32, name="e1")
            e1s = spool.tile([128, 2], F32, name="e1s")
            nc.vector.tensor_scalar(e1[:], sb["f1b"][:], bqb[:, 1:2], None,
                                    ALU.subtract)
            nc.scalar.activation(e1s[:], e1[:], AF.Square)
            nc.vector.tensor_scalar(e1s[:], e1s[:], -1.0, 1.0, ALU.mult, ALU.add)
            nc.vector.tensor_tensor(mask[:], mask[:], e1s[:], ALU.mult)
            dbg("mask", mask[:])

            act6f = apool.tile([128, 2, 32, 64], FP16, name="act6f",
                               tag="actfull")
            y6v = y6.rearrange("p mh i (y x) -> p mh i y x", y=8)
            a6fv = act6f.rearrange("p mh i (y x) -> p mh i y x", y=8)
            a6v = act6f.rearrange("p mh i (y x two) -> p mh i y x two",
                                  y=8, two=2)
            pl3 = apool.tile([128, 2, 32, 8, 4], FP16, name="pl3", tag="scr16")
            p3v = pl3.rearrange("p mh i (y two) x -> p mh i y two x", two=2)
            act6p = apool.tile([128, 2, 16, 32], FP16, name="act6p", tag="cparscr")
            a6pv = act6p.rearrange("p mh (y x) i -> p mh i y x", y=4)
            ps7 = [ppool.tile([128, 32], F32, name=f"ps7_{mh}", tag="ps", bufs=8)
                   for mh in range(4)]
            fc7e = ext["fc7s"]
            for h in range(2):
                s6, t6 = bn_affine_params(gs6[:, 2 * h:2 * h + 1],
                                          gs6[:, 2 * h + 1:2 * h + 2],
                                          sb["g6v"][:, h:h + 1],
                                          sb["be6v"][:, h:h + 1],
                                          N56, 128, f"L6h{h}",
                                          mask=mask[:, h:h + 1])
                nc.scalar.activation(a6fv[:, h], y6v[:, h], AF.Relu,
                                     bias=t6, scale=s6)
                nc.vector.tensor_tensor(pl3[:, h],
                                        a6v[:, h, :, :, :, 0:1].squeeze(4),
                                        a6v[:, h, :, :, :, 1:2].squeeze(4),
                                        ALU.max)
                nc.vector.tensor_tensor(a6pv[:, h],
                                        p3v[:, h, :, :, 0:1, :].squeeze(3),
                                        p3v[:, h, :, :, 1:2, :].squeeze(3),
                                        ALU.max)
                for pix in range(16):
                    kc = h * 16 + pix
                    w7 = spool.tile([128, 512], FP16, name="w7", tag="w7",
                                    bufs=3)
                    nc.sync.dma_start(w7[:], fc7e[:, kc, :])
                    rhs7 = act6p[:, h, pix, :]
                    for mh in range(4):
                        nc.tensor.matmul(ps7[mh][:],
                                         w7[:, mh * 128:(mh + 1) * 128],
                                         rhs7, start=(kc == 0), stop=(kc == 31))
            dbg("act6p", act6p[:])
            y7l = spool.tile([128, 4, 32], FP16, name="y7l")
            for mh in range(4):
                nc.scalar.copy(y7l[:, mh, :], ps7[mh][:])
            y7b = dpool.tile([512, 32], FP16, name="y7b")
            y7bv = y7b.rearrange("(mh p) b -> p mh b", mh=4)
            nc.sync.dma_start(y7bv[:], y7l[:])
            y7g = dpool.tile([4096, 32], FP16, name="y7g", addr_space="Shared")
            nc.gpsimd.collective_compute(
                "AllGather", ALU.bypass, replica_groups=REPLICA,
                ins=[y7b.opt()], outs=[y7g.opt()])
            y7gv = y7g.rearrange("(c mh p) b -> mh p c b", c=8, mh=4)
            act7 = apool.tile([128, 4, 256], FP16, name="act7", tag="cparscr")
            scr7 = spool.tile([128, 256], FP16, name="scr7", tag="fcscr")
            y7sb = spool.tile([128, 4, 8, 32], FP16, name="y7sb")
            ss7 = spool.tile([128, 2, 4], F32, name="ss7")
            for mh in range(4):
                nc.sync.dma_start(y7sb[:, mh], y7gv[mh])
                yv = y7sb[:, mh].rearrange("p a b -> p (a b)")
                nc.vector.tensor_scalar(scr7[:], yv, 0.0, 0.0, ALU.add,
                                        ALU.add, accum_out=ss7[:, 0, mh:mh + 1])
                nc.scalar.activation(scr7[:], yv, AF.Square,
                                     accum_out=ss7[:, 1, mh:mh + 1])
            s7, t7 = bn_affine_params(ss7[:, 0, :], ss7[:, 1, :],
                                      sb["g7v"][:], sb["be7v"][:],
                                      256, 128, "fc7", k=4)
            for mh in range(4):
                yv = y7sb[:, mh].rearrange("p a b -> p (a b)")
                nc.scalar.activation(act7[:, mh, :], yv, AF.Relu,
                                     bias=t7[:, mh:mh + 1],
                                     scale=s7[:, mh:mh + 1])
            dbg("act7", act7[:])

            # ================= FC8 =================
            ps8 = [ppool.tile([128, 256], F32, name=f"ps8_{mh}", tag="ps", bufs=8)
                   for mh in range(4)]
            for kc in range(4):
                for mh in range(4):
                    nc.tensor.matmul(ps8[mh][:],
                                     sb["fc8s"][:, kc, mh * 128:(mh + 1) * 128],
                                     act7[:, kc, :],
                                     start=(kc == 0), stop=(kc == 3))
            act8 = apool.tile([128, 4, 256], FP16, name="act8", tag="cparscr")
            ss8 = spool.tile([128, 2, 4], F32, name="ss8")
            for mh in range(4):
                nc.vector.tensor_scalar(scr7[:], ps8[mh][:], 0.0, 0.0, ALU.add,
                                        ALU.add, accum_out=ss8[:, 0, mh:mh + 1])
                nc.scalar.activation(scr7[:], ps8[mh][:], AF.Square,
                                     accum_out=ss8[:, 1, mh:mh + 1])
            s8, t8 = bn_affine_params(ss8[:, 0, :], ss8[:, 1, :],
                                      sb["g8v"][:], sb["be8v"][:],
                                      256, 128, "fc8", k=4)
            for mh in range(4):
                nc.scalar.activation(act8[:, mh, :], ps8[mh][:], AF.Relu,
                                     bias=t8[:, mh:mh + 1],
                                     scale=s8[:, mh:mh + 1])
            dbg("act8", act8[:])

            # ================= FC9 =================
            ps9 = ppool.tile([10, 256], F32, name="ps9", tag="ps", bufs=8)
            for kc in range(4):
                nc.tensor.matmul(ps9[:], sb["fc9s"][:, kc, :], act8[:, kc, :],
                                 start=(kc == 0), stop=(kc == 3))
            out_sb = spool.tile([10, 256], F32, name="out_sb")
            nc.vector.tensor_scalar_add(out_sb[:], ps9[:], sb["fc9bv"][:])
            nc.sync.dma_start(out_ext[:].transpose([1, 0]), out_sb[:])

    nc.compile()
    return nc, dbg_ext


_CACHE = {}


def _get_nc(debug_taps=()):
    key = tuple(sorted(debug_taps))
    if key not in _CACHE:
        _CACHE[key] = build_nc(debug_taps)
    return _CACHE[key]


def kernel(_debug_taps=(), _trace=False, **inputs):
    _install_ntff_hook()
    x1cols, shared = _host_prep(inputs)
    nc, dbg_ext = _get_nc(_debug_taps)
    in_maps = []
    for core in range(N_CORES):
        m = {"x1col": x1cols[core]}
        m.update(shared)
        in_maps.append(m)
    res = run_bass_kernel_spmd(nc, in_maps, core_ids=list(range(N_CORES)),
                               trace=_trace)
    out = res.results[0]["out"]
    if _debug_taps or _trace:
        return out, res
    return out


if __name__ == "__main__":
    rng = np.random.RandomState(0)
    ins = {"x": rng.randn(256, 3, 32, 32).astype(np.float32)}
    shapes = [(64, 3), (64, 64), (128, 64), (128, 128), (256, 128), (256, 256)]
    for i, (co, ci) in enumerate(shapes, start=1):
        ins[f"w{i}"] = (rng.randn(co, ci, 3, 3) * 0.05).astype(np.float32)
        ins[f"b{i}"] = np.zeros(co, np.float32)
        ins[f"g{i}"] = np.ones(co, np.float32)
        ins[f"be{i}"] = np.zeros(co, np.float32)
    ins["hash_a"] = rng.randn(2, 2306).astype(np.float32)
    ins["fc7_w"] = (rng.randn(512, 4096) * 0.02).astype(np.float32)
    ins["fc7_b"] = np.zeros(512, np.float32)
    ins["g7"] = np.ones(512, np.float32)
    ins["be7"] = np.zeros(512, np.float32)
    ins["fc8_w"] = (rng.randn(512, 512) * 0.02).astype(np.float32)
    ins["fc8_b"] = np.zeros(512, np.float32)
    ins["g8"] = np.ones(512, np.float32)
    ins["be8"] = np.zeros(512, np.float32)
    ins["fc9_w"] = (rng.randn(10, 512) * 0.02).astype(np.float32)
    ins["fc9_b"] = np.zeros(10, np.float32)
    out = kernel(**ins)
    print("out", out.shape, out.dtype, np.abs(out).mean())


# revision 32
# speedup vs baseline: 1.1096x; 1.1096x over previous
"""Bass/Trainium2 kernel for nn_ALSHVGGNet (8 NeuronCores, data parallel).

Strategy:
- Batch 256 sharded 32/core; all conv/fc weights replicated (host-prepped fp16
  layouts); fp16 matmuls with f32 PSUM accumulation.
- BatchNorm uses full-batch statistics: per-layer per-channel (sum, sumsq)
  computed on-device via bn_stats and AllReduce'd across the 8 cores.
- Conv1/Conv2 (64 ch) run in a (parity, channel) packed layout: 2 images share
  the 128 partitions via block-diagonal weights so DVE/ACT epilogues use all
  128 lanes.
- Conv3 pairs taps along K (shifted activation copy on partitions 64..127).
- ALSH mask: filter codes precomputed on host from w6/hash_a (weights only);
  query code from the all-reduced act5 channel sums (sign-invariant
  simplification of the reference math); mask folded into BN6's affine.
- FC stack: act6 pooled activations AllGather'd, then every core computes the
  full-batch FC7/8/9 locally (BN7/8 stats become core-local).
"""

import os
import sys
import types

sys.path.insert(0, "/opt/trn_rl_repo")

import numpy as np

import concourse.bass as bass
import concourse.mybir as mybir
import concourse.tile as tile
from concourse import bacc
from concourse.bass_utils import run_bass_kernel_spmd

N_CORES = 8
SHARD = 32          # images per core
EPS = 1e-5
U = 0.999
F32 = mybir.dt.float32
FP16 = mybir.dt.float16
AX = mybir.AxisListType
ALU = mybir.AluOpType
AF = mybir.ActivationFunctionType

# global counts for BN stats normalization
N12 = 256 * 1024    # layers 1,2
N34 = 256 * 256     # layers 3,4
N56 = 256 * 64      # layers 5,6


def _install_ntff_hook():
    """Best effort registration of the axon NTFF profile hook (timing only)."""
    try:
        import antenv
        from trn_agent_boot.trn_boot import _ntff_profile_via_ctypes

        hooks = types.ModuleType("antenv.axon_hooks")
        hook = _ntff_profile_via_ctypes("/opt/axon/libaxon_pjrt.so")
        hooks.get_axon_ntff_profile_hook = lambda: hook
        hooks.set_axon_ntff_profile_hook = lambda h: None
        sys.modules["antenv.axon_hooks"] = hooks
        antenv.axon_hooks = hooks
    except Exception:
        pass


# ---------------------------------------------------------------------------
# Host-side input preparation
# ---------------------------------------------------------------------------

def _host_prep(inputs):
    """Build per-core and shared device input arrays from the raw inputs."""
    f16 = np.float16
    d = {}

    x = np.asarray(inputs["x"], np.float32)           # (256, 3, 32, 32)
    B = x.shape[0]
    assert B == N_CORES * SHARD

    # --- x im2col in (parity-block, tap, ci) x (pair, pix) layout ----------
    xp = np.zeros((B, 3, 34, 34), np.float32)
    xp[:, :, 1:33, 1:33] = x
    x1cols = []
    for core in range(N_CORES):
        sh = xp[core * SHARD:(core + 1) * SHARD]      # (32, 3, 34, 34)
        col = np.zeros((2, 9, 3, 16, 1024), np.float32)
        for dy in range(3):
            for dx in range(3):
                w = sh[:, :, dy:dy + 32, dx:dx + 32]  # (32, 3, 32, 32)
                w = w.reshape(16, 2, 3, 1024)
                col[:, dy * 3 + dx] = w.transpose(1, 2, 0, 3)
        x1cols.append(np.ascontiguousarray(
            col.reshape(54, 16 * 1024)).astype(f16))

    def w_tap(w):  # (O, I, 3, 3) -> [tap][I, O]
        return [np.ascontiguousarray(w[:, :, t // 3, t % 3].T) for t in range(9)]

    w1 = np.asarray(inputs["w1"], np.float32)
    w2 = np.asarray(inputs["w2"], np.float32)
    w3 = np.asarray(inputs["w3"], np.float32)
    w4 = np.asarray(inputs["w4"], np.float32)
    w5 = np.asarray(inputs["w5"], np.float32)
    w6 = np.asarray(inputs["w6"], np.float32)

    # L1 block-diag [54, 128]
    w1bd = np.zeros((54, 128), np.float32)
    for t in range(9):
        blk = w1[:, :, t // 3, t % 3].T               # (3 ci, 64 co)
        for par in range(2):
            w1bd[par * 27 + t * 3:par * 27 + t * 3 + 3, par * 64:par * 64 + 64] = blk
    d["w1bd"] = w1bd.astype(f16)

    # L2 block-diag per tap [128, 9, 128]
    w2t = w_tap(w2)
    w2bd = np.zeros((128, 9, 128), np.float32)
    for t in range(9):
        for par in range(2):
            w2bd[par * 64:par * 64 + 64, t, par * 64:par * 64 + 64] = w2t[t]
    d["w2bd"] = w2bd.astype(f16)

    # L3 tap-paired passes [128, 6, 128]: rows (s*64+ci)
    w3t = w_tap(w3)
    w3p = np.zeros((128, 6, 128), np.float32)
    for dy in range(3):
        w3p[0:64, 2 * dy, :] = w3t[dy * 3 + 0]
        w3p[64:128, 2 * dy, :] = w3t[dy * 3 + 1]
        w3p[0:64, 2 * dy + 1, :] = w3t[dy * 3 + 2]
    d["w3p"] = w3p.astype(f16)

    d["w4s"] = np.stack(w_tap(w4), axis=1).astype(f16)          # [128, 9, 128]
    d["w5s"] = np.stack(w_tap(w5), axis=1).astype(f16)          # [128, 9, 256]
    w6s = np.zeros((128, 9, 2, 256), np.float32)
    for t in range(9):
        wt = w6[:, :, t // 3, t % 3].T                           # (256 ci, 256 co)
        w6s[:, t, 0, :] = wt[0:128]
        w6s[:, t, 1, :] = wt[128:256]
    d["w6s"] = w6s.astype(f16)

    fc7 = np.asarray(inputs["fc7_w"], np.float32)                # (512, 4096)
    d["fc7s"] = np.ascontiguousarray(
        fc7.reshape(512, 2, 128, 16).transpose(2, 1, 3, 0)
        .reshape(128, 32, 512)).astype(f16)
    fc8 = np.asarray(inputs["fc8_w"], np.float32)                # (512, 512)
    d["fc8s"] = np.ascontiguousarray(
        fc8.T.reshape(4, 128, 512).transpose(1, 0, 2)).astype(f16)
    fc9 = np.asarray(inputs["fc9_w"], np.float32)                # (10, 512)
    d["fc9s"] = np.ascontiguousarray(
        fc9.T.reshape(4, 128, 10).transpose(1, 0, 2)).astype(f16)
    d["fc9bv"] = np.asarray(inputs["fc9_b"], np.float32).reshape(10, 1)

    # gamma/beta in stat layouts (f32)
    for i, shape in [(1, (64, 1)), (2, (64, 1)), (3, (128, 1)), (4, (128, 1))]:
        d[f"g{i}v"] = np.asarray(inputs[f"g{i}"], np.float32).reshape(shape)
        d[f"be{i}v"] = np.asarray(inputs[f"be{i}"], np.float32).reshape(shape)
    for i in (5, 6):
        d[f"g{i}v"] = np.ascontiguousarray(
            np.asarray(inputs[f"g{i}"], np.float32).reshape(2, 128).T)
        d[f"be{i}v"] = np.ascontiguousarray(
            np.asarray(inputs[f"be{i}"], np.float32).reshape(2, 128).T)
    for i in (7, 8):
        d[f"g{i}v"] = np.ascontiguousarray(
            np.asarray(inputs[f"g{i}"], np.float32).reshape(4, 128).T)
        d[f"be{i}v"] = np.ascontiguousarray(
            np.asarray(inputs[f"be{i}"], np.float32).reshape(4, 128).T)

    # fold/broadcast helpers for the (parity, channel) layers
    fold = np.zeros((128, 64), np.float32)
    bc = np.zeros((64, 128), np.float32)
    for c in range(64):
        fold[c, c] = fold[64 + c, c] = 1.0
        bc[c, c] = bc[c, 64 + c] = 1.0
    d["fold64"] = fold
    d["bc64"] = bc
    d["ones2"] = np.ones((2, 128), np.float32)
    d["id2"] = np.eye(2, dtype=np.float32)

    # --- ALSH host precompute (weights only) -------------------------------
    hash_a = np.asarray(inputs["hash_a"], np.float32)            # (2, 2306)
    wf = w6.reshape(256, -1)
    norms = np.linalg.norm(wf, axis=1)
    wf_s = wf * (U / norms.max())
    ns = np.linalg.norm(wf_s, axis=1)
    P = np.concatenate([wf_s, (ns ** 2)[:, None], (ns ** 4)[:, None]], axis=1)
    bits_f = (P @ hash_a.T) > 0                                   # (256, 2)
    f0 = bits_f[:, 0].astype(np.float32).reshape(2, 128).T        # [128, 2(mh)]
    f1 = bits_f[:, 1].astype(np.float32).reshape(2, 128).T
    d["f0b"] = np.ascontiguousarray(f0)
    d["f1b"] = np.ascontiguousarray(f1)
    ha9 = hash_a[:, :2304].reshape(2, 9, 256).sum(1)              # (2 bits, 256 c)
    ha9m = np.zeros((128, 2, 2), np.float32)                      # (p, mh, j)
    for mh in range(2):
        ha9m[:, mh, :] = ha9[:, mh * 128:(mh + 1) * 128].T
    d["ha9m"] = ha9m

    shared = d
    return x1cols, shared


SHARED_SPECS = {
    "w1bd": ((54, 128), FP16), "w2bd": ((128, 9, 128), FP16),
    "w3p": ((128, 6, 128), FP16), "w4s": ((128, 9, 128), FP16),
    "w5s": ((128, 9, 256), FP16), "w6s": ((128, 9, 2, 256), FP16),
    "fc7s": ((128, 32, 512), FP16), "fc8s": ((128, 4, 512), FP16),
    "fc9s": ((128, 4, 10), FP16), "fc9bv": ((10, 1), F32),
    "g1v": ((64, 1), F32), "be1v": ((64, 1), F32),
    "g2v": ((64, 1), F32), "be2v": ((64, 1), F32),
    "g3v": ((128, 1), F32), "be3v": ((128, 1), F32),
    "g4v": ((128, 1), F32), "be4v": ((128, 1), F32),
    "g5v": ((128, 2), F32), "be5v": ((128, 2), F32),
    "g6v": ((128, 2), F32), "be6v": ((128, 2), F32),
    "g7v": ((128, 4), F32), "be7v": ((128, 4), F32),
    "g8v": ((128, 4), F32), "be8v": ((128, 4), F32),
    "fold64": ((128, 64), F32), "bc64": ((64, 128), F32),
    "ones2": ((2, 128), F32), "id2": ((2, 2), F32),
    "f0b": ((128, 2), F32), "f1b": ((128, 2), F32),
    "ha9m": ((128, 2, 2), F32),
}

REPLICA = [list(range(N_CORES))]


def build_nc(debug_taps=()):
    nc = bacc.Bacc("TRN2", target_bir_lowering=False, debug=False,
                   num_devices=N_CORES)

    x1col_ext = nc.dram_tensor("x1col", [54, 16384], FP16, kind="ExternalInput")
    ext = {}
    for name, (shape, dt) in SHARED_SPECS.items():
        ext[name] = nc.dram_tensor(name, list(shape), dt, kind="ExternalInput")
    out_ext = nc.dram_tensor("out", [256, 10], F32, kind="ExternalOutput")
    dbg_ext = {}

    with tile.TileContext(nc) as tc:
        with (
            tc.tile_pool(name="const", bufs=1) as cpool,
            tc.tile_pool(name="acts", bufs=1) as apool,
            tc.tile_pool(name="scr", bufs=1) as spool,
            tc.tile_pool(name="psum", bufs=1, space="PSUM") as ppool,
            tc.tile_pool(name="dram", bufs=1, space="DRAM") as dpool,
        ):
            # ---- persistent consts/weights in SBUF (fc7s streamed later) ----
            sb = {}
            for name, (shape, dt) in SHARED_SPECS.items():
                if name == "fc7s":
                    continue
                t = cpool.tile(list(shape), dt, name=f"sb_{name}")
                nc.sync.dma_start(t[:], ext[name][:])
                sb[name] = t

            def dbg(name, ap):
                if name in debug_taps:
                    sh = [int(s) for s in ap.shape]
                    dt = ap.dtype
                    dbg_ext[name] = nc.dram_tensor(f"dbg_{name}", sh, dt,
                                                   kind="ExternalOutput")
                    nc.sync.dma_start(dbg_ext[name][:], ap)

            # ---- small helper chains ----
            def stat_combine(statv, T, half_cnt, name):
                """statv: [128, T, 6] bn_stats rows -> sums [128, 2] (sum, sumsq).

                half_cnt = per-tile even/odd element count (FD/2).
                """
                sm = spool.tile([128, 4], F32, name=f"sm_{name}")
                # sum of means (even + odd)
                nc.vector.tensor_reduce(sm[:, 0:1], statv[:, :, 1:2].squeeze(2),
                                        AX.X, ALU.add)
                nc.vector.tensor_reduce(sm[:, 1:2], statv[:, :, 4:5].squeeze(2),
                                        AX.X, ALU.add)
                # sum of count*var
                nc.vector.tensor_reduce(sm[:, 2:3], statv[:, :, 2:3].squeeze(2),
                                        AX.X, ALU.add)
                nc.vector.tensor_reduce(sm[:, 3:4], statv[:, :, 5:6].squeeze(2),
                                        AX.X, ALU.add)
                # sum of means^2
                msq = spool.tile([128, 2 * T], F32, name=f"msq_{name}")
                nc.vector.tensor_tensor(msq[:, 0:T], statv[:, :, 1:2].squeeze(2),
                                        statv[:, :, 1:2].squeeze(2), ALU.mult)
                nc.vector.tensor_tensor(msq[:, T:2 * T], statv[:, :, 4:5].squeeze(2),
                                        statv[:, :, 4:5].squeeze(2), ALU.mult)
                m2 = spool.tile([128, 1], F32, name=f"m2_{name}")
                nc.vector.tensor_reduce(m2[:], msq[:], AX.X, ALU.add)

                sums = spool.tile([128, 2], F32, name=f"sums_{name}")
                # sum = half_cnt * (sm0 + sm1)
                t0 = spool.tile([128, 1], F32, name=f"t0_{name}")
                nc.vector.tensor_tensor(t0[:], sm[:, 0:1], sm[:, 1:2], ALU.add)
                nc.vector.tensor_scalar_mul(sums[:, 0:1], t0[:], float(half_cnt))
                # sumsq = sm2 + sm3 + half_cnt * m2
                t1 = spool.tile([128, 1], F32, name=f"t1_{name}")
                nc.vector.tensor_tensor(t1[:], sm[:, 2:3], sm[:, 3:4], ALU.add)
                nc.vector.tensor_scalar(sums[:, 1:2], m2[:], float(half_cnt),
                                        None, ALU.mult)
                nc.vector.tensor_tensor(sums[:, 1:2], sums[:, 1:2], t1[:], ALU.add)
                return sums

            def allreduce(sums_ap, cols, name):
                ib = dpool.tile([128, cols], F32, name=f"arin_{name}")
                ob = dpool.tile([128, cols], F32, name=f"arout_{name}")
                nc.sync.dma_start(ib[:], sums_ap)
                nc.gpsimd.collective_compute(
                    "AllReduce", ALU.add, replica_groups=REPLICA,
                    ins=[ib.opt()], outs=[ob.opt()])
                g = spool.tile([128, cols], F32, name=f"gsum_{name}")
                nc.sync.dma_start(g[:], ob[:])
                return g

            def bn_affine_params(S, Q, g_ap, be_ap, n_total, P_, name,
                                 mask=None, k=1):
                """S,Q: [P_,k] global sum/sumsq -> (s, t) [P_,k] f32 tiles."""
                st = spool.tile([P_, 8 * k], F32, name=f"st_{name}")
                m, v, w, r0, a, dtmp, s_t, t_t = [st[:, i * k:(i + 1) * k]
                                                  for i in range(8)]
                nc.vector.tensor_scalar_mul(m, S, 1.0 / n_total)
                msq = spool.tile([P_, k], F32, name=f"stm_{name}")
                nc.vector.tensor_tensor(msq, m, m, ALU.mult)
                nc.vector.tensor_scalar_mul(v, Q, 1.0 / n_total)
                nc.vector.tensor_tensor(v, v, msq, ALU.subtract)
                if mask is not None:
                    nc.vector.tensor_tensor(m, m, mask, ALU.mult)
                    nc.vector.tensor_tensor(v, v, mask, ALU.mult)
                nc.vector.tensor_scalar_add(v, v, EPS)   # v := var + eps
                nc.vector.reciprocal(w, v)
                nc.scalar.activation(r0, w, AF.Sqrt)
                # Newton step: r1 = r0 * (1.5 - 0.5 * v * r0^2)
                nc.vector.tensor_tensor(a, r0, r0, ALU.mult)
                nc.vector.tensor_tensor(a, a, v, ALU.mult)
                nc.vector.tensor_scalar(a, a, -0.5, 1.5, ALU.mult, ALU.add)
                nc.vector.tensor_tensor(r0, r0, a, ALU.mult)
                nc.vector.tensor_tensor(s_t, g_ap, r0, ALU.mult)
                if mask is not None:
                    nc.vector.tensor_tensor(s_t, s_t, mask, ALU.mult)
                nc.vector.tensor_tensor(dtmp, m, s_t, ALU.mult)
                nc.vector.tensor_tensor(t_t, be_ap, dtmp, ALU.subtract)
                return s_t, t_t

            def fold_bcast(sums, g_ap, be_ap, n_total, name):
                """(parity, channel) stats: fold to 64, bn math, broadcast to 128."""
                up = spool.tile([64, 2], F32, name=f"up_{name}")
                nc.vector.tensor_copy(up[:], sums[64:128, :])
                s64 = spool.tile([64, 2], F32, name=f"s64_{name}")
                nc.vector.tensor_tensor(s64[:], sums[0:64, :], up[:], ALU.add)
                s_t, t_t = bn_affine_params(s64[:, 0:1], s64[:, 1:2],
                                            g_ap, be_ap, n_total, 64, name)
                st128 = spool.tile([128, 2], F32, name=f"stb_{name}")
                nc.vector.tensor_copy(st128[0:64, 0:1], s_t)
                nc.vector.tensor_copy(st128[0:64, 1:2], t_t)
                nc.vector.tensor_copy(st128[64:128, :], st128[0:64, :])
                return st128[:, 0:1], st128[:, 1:2]

            # ================= Layer 1 =================
            y1 = apool.tile([128, 16384], FP16, name="y1", tag="ybuf")
            stat1 = spool.tile([128, 32, 6], F32, name="stat1")
            for t in range(32):
                if t % 8 == 0:
                    x1t = spool.tile([54, 8, 512], FP16, name="x1t", tag="x1t",
                                     bufs=3)
                    nc.gpsimd.dma_start(
                        x1t[:], x1col_ext[:, t * 512:(t + 8) * 512]
                        .rearrange("p (a b) -> p a b", a=8))
                ps = ppool.tile([128, 512], F32, name=f"ps1_{t}", tag="ps", bufs=8)
                nc.tensor.matmul(ps[:], sb["w1bd"][:], x1t[:, t % 8, :],
                                 start=True, stop=True)
                nc.vector.bn_stats(stat1[:, t, :], ps[:])
                nc.scalar.copy(y1[:, t * 512:(t + 1) * 512], ps[:])
            dbg("y1", y1[:])
            sums1 = stat_combine(stat1, 32, 256, "L1")
            gs1 = allreduce(sums1[:], 2, "L1")
            s1, t1 = fold_bcast(gs1, sb["g1v"][:], sb["be1v"][:], N12, "L1")

            act1 = apool.tile([128, 16, 34, 34], FP16, name="act1", tag="actpad")
            nc.gpsimd.memset(act1[:, :, 0:1, :], 0.0)
            nc.gpsimd.memset(act1[:, :, 33:34, :], 0.0)
            nc.gpsimd.memset(act1[:, :, 1:33, 0:1], 0.0)
            nc.gpsimd.memset(act1[:, :, 1:33, 33:34], 0.0)
            y1v = y1.rearrange("p (pr y x) -> p pr y x", pr=16, y=32, x=32)
            for p0, pn in [(0, 1), (1, 3), (4, 4), (8, 4), (12, 4)]:
                nc.scalar.activation(act1[:, p0:p0 + pn, 1:33, 1:33],
                                     y1v[:, p0:p0 + pn], AF.Relu,
                                     bias=t1, scale=s1)
            dbg("act1", act1[:])

            # ================= Layer 2 =================
            y2 = apool.tile([128, 16384], FP16, name="y2", tag="ybuf")
            stat2 = spool.tile([128, 32, 6], F32, name="stat2")
            for pr in range(16):
                pss = [ppool.tile([128, 512], F32, name=f"ps2_{pr}_{h}",
                                  tag="ps", bufs=8) for h in range(2)]
                for t in range(9):
                    dy, dx = t // 3, t % 3
                    for h in range(2):
                        rhs = act1[:, pr, h * 16 + dy:h * 16 + dy + 16,
                                   dx:dx + 32]
                        nc.tensor.matmul(pss[h][:], sb["w2bd"][:, t, :], rhs,
                                         start=(t == 0), stop=(t == 8))
                for h in range(2):
                    ti = pr * 2 + h
                    nc.vector.bn_stats(stat2[:, ti, :], pss[h][:])
                    nc.scalar.copy(y2[:, ti * 512:(ti + 1) * 512], pss[h][:])
            dbg("y2", y2[:])
            sums2 = stat_combine(stat2, 32, 256, "L2")
            gs2 = allreduce(sums2[:], 2, "L2")
            s2, t2 = fold_bcast(gs2, sb["g2v"][:], sb["be2v"][:], N12, "L2")

            act2f = apool.tile([128, 16, 1024], FP16, name="act2f", tag="actfull")
            y2v = y2.rearrange("p (pr q) -> p pr q", pr=16)
            a2v = act2f.rearrange("p pr (y x two) -> p pr y x two", y=32, two=2)
            pl1 = apool.tile([128, 16, 32, 16], FP16, name="pl1", tag="scr16")
            p1v = pl1.rearrange("p pr (y two) x -> p pr y two x", two=2)
            cpar = apool.tile([128, 16, 256], FP16, name="cpar", tag="cparscr")
            cpv = cpar.rearrange("p pr (y x) -> p pr y x", y=16)

            # scatter into act2p [128=(s,c), 32 img, 18, 18] with shift copy
            act2p = apool.tile([128, 32, 18, 18], FP16, name="act2p", tag="actp")
            nc.gpsimd.memset(act2p[:, :, 0:1, :], 0.0)
            nc.gpsimd.memset(act2p[:, :, 17:18, :], 0.0)
            nc.gpsimd.memset(act2p[0:64, :, 1:17, 0:1], 0.0)
            nc.gpsimd.memset(act2p[0:64, :, 1:17, 17:18], 0.0)
            nc.gpsimd.memset(act2p[64:128, :, 1:17, 16:18], 0.0)
            a2pv = act2p.rearrange("p (i ip) y x -> p i ip y x", ip=2)
            cp4 = cpar.rearrange("p pr (y x) -> p pr y x", y=16)
            for p0 in range(0, 16, 4):
                sl = slice(p0, p0 + 4)
                nc.scalar.activation(act2f[:, sl, :], y2v[:, sl],
                                     AF.Relu, bias=t2, scale=s2)
                nc.vector.tensor_tensor(pl1[:, sl],
                                        a2v[:, sl, :, :, 0:1].squeeze(4),
                                        a2v[:, sl, :, :, 1:2].squeeze(4),
                                        ALU.max)
                nc.vector.tensor_tensor(cpv[:, sl],
                                        p1v[:, sl, :, 0:1, :].squeeze(3),
                                        p1v[:, sl, :, 1:2, :].squeeze(3),
                                        ALU.max)
                nc.vector.tensor_copy(a2pv[0:64, sl, 0, 1:17, 1:17],
                                      cp4[0:64, sl])
                nc.gpsimd.tensor_copy(a2pv[64:128, sl, 1, 1:17, 0:16],
                                      cp4[64:128, sl])
                for pr in range(p0, p0 + 4):
                    nc.gpsimd.dma_start(a2pv[0:64, pr, 1, 1:17, 1:17],
                                        cp4[64:128, pr])
                    nc.gpsimd.dma_start(a2pv[64:128, pr, 0, 1:17, 0:16],
                                        cp4[0:64, pr])
            dbg("act2p", act2p[:])

            # ================= Layer 3 =================
            y3 = apool.tile([128, 32, 256], FP16, name="y3", tag="ybuf")
            stat3 = spool.tile([128, 16, 6], F32, name="stat3")
            passes = [(0, 0), (0, 2), (1, 0), (1, 2), (2, 0), (2, 2)]
            for tg in range(8):
                pss = [ppool.tile([128, 512], F32, name=f"ps3_{tg}_{u}",
                                  tag="ps", bufs=8) for u in range(2)]
                for pi, (dy, dx) in enumerate(passes):
                    for u in range(2):
                        i0 = (tg * 2 + u) * 2
                        rhs = act2p[:, i0:i0 + 2, dy:dy + 16, dx:dx + 16]
                        nc.tensor.matmul(pss[u][:],
                                         sb["w3p"][:, 2 * dy + (dx // 2), :],
                                         rhs, start=(pi == 0), stop=(pi == 5))
                for u in range(2):
                    ti = tg * 2 + u
                    i0 = ti * 2
                    nc.vector.bn_stats(stat3[:, ti, :], pss[u][:])
                    nc.scalar.copy(
                        y3[:, i0:i0 + 2, :].rearrange("p a b -> p (a b)"),
                        pss[u][:])
            dbg("y3", y3[:])
            sums3 = stat_combine(stat3, 16, 256, "L3")
            gs3 = allreduce(sums3[:], 2, "L3")
            s3, t3 = bn_affine_params(gs3[:, 0:1], gs3[:, 1:2], sb["g3v"][:],
                                      sb["be3v"][:], N34, 128, "L3")

            act3 = apool.tile([128, 32, 18, 18], FP16, name="act3", tag="actpad")
            nc.gpsimd.memset(act3[:, :, 0:1, :], 0.0)
            nc.gpsimd.memset(act3[:, :, 17:18, :], 0.0)
            nc.gpsimd.memset(act3[:, :, 1:17, 0:1], 0.0)
            nc.gpsimd.memset(act3[:, :, 1:17, 17:18], 0.0)
            y3v = y3.rearrange("p i (y x) -> p i y x", y=16)
            for i0, ni in [(0, 2), (2, 6), (8, 8), (16, 8), (24, 8)]:
                nc.scalar.activation(act3[:, i0:i0 + ni, 1:17, 1:17],
                                     y3v[:, i0:i0 + ni], AF.Relu,
                                     bias=t3, scale=s3)
            dbg("act3", act3[:])

            # ================= Layer 4 =================
            y4 = apool.tile([128, 32, 256], FP16, name="y4", tag="ybuf")
            stat4 = spool.tile([128, 16, 6], F32, name="stat4")
            for tg in range(8):
                pss = [ppool.tile([128, 512], F32, name=f"ps4_{tg}_{u}",
                                  tag="ps", bufs=8) for u in range(2)]
                for t in range(9):
                    dy, dx = t // 3, t % 3
                    for u in range(2):
                        i0 = (tg * 2 + u) * 2
                        rhs = act3[:, i0:i0 + 2, dy:dy + 16, dx:dx + 16]
                        nc.tensor.matmul(pss[u][:], sb["w4s"][:, t, :], rhs,
                                         start=(t == 0), stop=(t == 8))
                for u in range(2):
                    ti = tg * 2 + u
                    i0 = ti * 2
                    nc.vector.bn_stats(stat4[:, ti, :], pss[u][:])
                    nc.scalar.copy(
                        y4[:, i0:i0 + 2, :].rearrange("p a b -> p (a b)"),
                        pss[u][:])
            dbg("y4", y4[:])
            sums4 = stat_combine(stat4, 16, 256, "L4")
            gs4 = allreduce(sums4[:], 2, "L4")
            s4, t4 = bn_affine_params(gs4[:, 0:1], gs4[:, 1:2], sb["g4v"][:],
                                      sb["be4v"][:], N34, 128, "L4")

            act4f = apool.tile([128, 32, 256], FP16, name="act4f", tag="actfull")
            y4v = y4.rearrange("p i (y x) -> p i y x", y=16)
            a4fv = act4f.rearrange("p i (y x) -> p i y x", y=16)
            a4v = act4f.rearrange("p i (y x two) -> p i y x two", y=16, two=2)
            pl2 = apool.tile([128, 32, 16, 8], FP16, name="pl2", tag="scr16")
            p2v = pl2.rearrange("p i (y two) x -> p i y two x", two=2)
            for i0 in range(0, 32, 8):
                sl = slice(i0, i0 + 8)
                nc.scalar.activation(a4fv[:, sl], y4v[:, sl],
                                     AF.Relu, bias=t4, scale=s4)
                nc.vector.tensor_tensor(pl2[:, sl],
                                        a4v[:, sl, :, :, 0:1].squeeze(4),
                                        a4v[:, sl, :, :, 1:2].squeeze(4),
                                        ALU.max)
            act4p = apool.tile([128, 32, 10, 10], FP16, name="act4p", tag="actp")
            nc.gpsimd.memset(act4p[:, :, 0:1, :], 0.0)
            nc.gpsimd.memset(act4p[:, :, 9:10, :], 0.0)
            nc.gpsimd.memset(act4p[:, :, 1:9, 0:1], 0.0)
            nc.gpsimd.memset(act4p[:, :, 1:9, 9:10], 0.0)
            for i0 in range(0, 32, 8):
                sl = slice(i0, i0 + 8)
                nc.vector.tensor_tensor(act4p[:, sl, 1:9, 1:9],
                                        p2v[:, sl, :, 0:1, :].squeeze(3),
                                        p2v[:, sl, :, 1:2, :].squeeze(3),
                                        ALU.max)
            dbg("act4p", act4p[:])

            # ================= Layer 5 =================
            y5 = apool.tile([128, 2, 32, 64], FP16, name="y5", tag="ybuf")
            stat5 = spool.tile([128, 2, 4, 6], F32, name="stat5")
            for mh in range(2):
                for tg in range(2):
                    pss = [ppool.tile([128, 512], F32, name=f"ps5_{mh}_{tg}_{u}",
                                      tag="ps", bufs=8) for u in range(2)]
                    for t in range(9):
                        dy, dx = t // 3, t % 3
                        for u in range(2):
                            i0 = (tg * 2 + u) * 8
                            rhs = act4p[:, i0:i0 + 8, dy:dy + 8, dx:dx + 8]
                            nc.tensor.matmul(
                                pss[u][:], sb["w5s"][:, t, mh * 128:(mh + 1) * 128],
                                rhs, start=(t == 0), stop=(t == 8))
                    for u in range(2):
                        ti = tg * 2 + u
                        i0 = ti * 8
                        nc.vector.bn_stats(stat5[:, mh, ti, :], pss[u][:])
                        nc.scalar.copy(
                            y5[:, mh, i0:i0 + 8, :].rearrange("p a b -> p (a b)"),
                            pss[u][:])
            dbg("y5", y5[:])
            sums5 = spool.tile([128, 4], F32, name="sums5")
            for h in range(2):
                sums5h = stat_combine(stat5[:, h], 4, 256, f"L5h{h}")
                nc.vector.tensor_copy(sums5[:, 2 * h:2 * h + 2], sums5h[:])
            gs5 = allreduce(sums5[:], 4, "L5")

            act5 = apool.tile([128, 2, 32, 10, 10], FP16, name="act5",
                              tag="actpad")
            for h in range(2):
                nc.gpsimd.memset(act5[:, h, :, 0:1, :], 0.0)
                nc.gpsimd.memset(act5[:, h, :, 9:10, :], 0.0)
                nc.gpsimd.memset(act5[:, h, :, 1:9, 0:1], 0.0)
                nc.gpsimd.memset(act5[:, h, :, 1:9, 9:10], 0.0)
            qsum = spool.tile([128, 2], F32, name="qsum")
            y5v = y5.rearrange("p mh i (y x) -> p mh i y x", y=8)
            for h in range(2):
                s5, t5 = bn_affine_params(gs5[:, 2 * h:2 * h + 1],
                                          gs5[:, 2 * h + 1:2 * h + 2],
                                          sb["g5v"][:, h:h + 1],
                                          sb["be5v"][:, h:h + 1],
                                          N56, 128, f"L5h{h}")
                nc.scalar.activation(act5[:, h, :, 1:9, 1:9], y5v[:, h],
                                     AF.Relu, bias=t5, scale=s5,
                                     accum_out=qsum[:, h:h + 1])
            dbg("act5", act5[:])

            # ================= Layer 6 =================
            # All 8 psum tiles open; K-chunk h=0 matmuls run for every tile
            # first, so the h=1 chunk (which needs act5 h1 / AR5b) comes last.
            y6 = apool.tile([128, 2, 32, 64], FP16, name="y6", tag="scr16")
            stat6 = spool.tile([128, 2, 4, 6], F32, name="stat6")
            ps6 = {}
            for mh in range(2):
                for ti in range(4):
                    ps6[(mh, ti)] = ppool.tile([128, 512], F32,
                                               name=f"ps6_{mh}_{ti}",
                                               tag="ps", bufs=8)
            for h in range(2):
                for t in range(9):
                    dy, dx = t // 3, t % 3
                    for mh in range(2):
                        for ti in range(4):
                            i0 = ti * 8
                            rhs = act5[:, h, i0:i0 + 8, dy:dy + 8, dx:dx + 8]
                            nc.tensor.matmul(
                                ps6[(mh, ti)][:],
                                sb["w6s"][:, t, h, mh * 128:(mh + 1) * 128],
                                rhs, start=(h == 0 and t == 0),
                                stop=(h == 1 and t == 8))
            for mh in range(2):
                for ti in range(4):
                    i0 = ti * 8
                    nc.vector.bn_stats(stat6[:, mh, ti, :], ps6[(mh, ti)][:])
                    nc.scalar.copy(
                        y6[:, mh, i0:i0 + 8, :].rearrange("p a b -> p (a b)"),
                        ps6[(mh, ti)][:])
            dbg("y6", y6[:])
            sums6 = spool.tile([128, 6], F32, name="sums6")
            for h in range(2):
                sums6h = stat_combine(stat6[:, h], 4, 256, f"L6h{h}")
                nc.vector.tensor_copy(sums6[:, 2 * h:2 * h + 2], sums6h[:])
            nc.vector.tensor_copy(sums6[:, 4:6], qsum[:])
            gs6 = allreduce(sums6[:], 6, "L6")

            # ---- ALSH mask from global qsums ----
            psd = ppool.tile([2, 2], F32, name="psd", tag="ps", bufs=8)
            for mh in range(2):
                nc.tensor.matmul(psd[:, 0:1], sb["ha9m"][:, mh, :],
                                 gs6[:, 4 + mh:5 + mh],
                                 start=(mh == 0), stop=(mh == 1))
            dsb = spool.tile([2, 2], F32, name="dsb")
            nc.vector.tensor_copy(dsb[:, 0:1], psd[:, 0:1])
            bq = spool.tile([2, 2], F32, name="bq")
            nc.vector.tensor_scalar(bq[:, 0:1], dsb[:, 0:1], 0.0, None, ALU.is_gt)
            bqd = spool.tile([2, 2], F32, name="bqd")
            nc.vector.tensor_scalar_mul(bqd[:], sb["id2"][:], bq[:, 0:1])
            psb2 = ppool.tile([128, 2], F32, name="psb2", tag="ps", bufs=8)
            nc.tensor.matmul(psb2[:], sb["ones2"][:], bqd[:],
                             start=True, stop=True)
            bqb = spool.tile([128, 2], F32, name="bqb")
            nc.vector.tensor_copy(bqb[:], psb2[:])
            mask = spool.tile([128, 2], F32, name="mask")
            e0 = spool.tile([128, 2], F32, name="e0")
            e0s = spool.tile([128, 2], F32, name="e0s")
            nc.vector.tensor_scalar(e0[:], sb["f0b"][:], bqb[:, 0:1], None,
                                    ALU.subtract)
            nc.scalar.activation(e0s[:], e0[:], AF.Square)
            nc.vector.tensor_scalar(mask[:], e0s[:], -1.0, 1.0, ALU.mult, ALU.add)
            e1 = spool.tile([128, 2], F32, name="e1")
            e1s = spool.tile([128, 2], F32, name="e1s")
            nc.vector.tensor_scalar(e1[:], sb["f1b"][:], bqb[:, 1:2], None,
                                    ALU.subtract)
            nc.scalar.activation(e1s[:], e1[:], AF.Square)
            nc.vector.tensor_scalar(e1s[:], e1s[:], -1.0, 1.0, ALU.mult, ALU.add)
            nc.vector.tensor_tensor(mask[:], mask[:], e1s[:], ALU.mult)
            dbg("mask", mask[:])

            act6f = apool.tile([128, 2, 32, 64], FP16, name="act6f",
                               tag="actfull")
            y6v = y6.rearrange("p mh i (y x) -> p mh i y x", y=8)
            a6fv = act6f.rearrange("p mh i (y x) -> p mh i y x", y=8)
            a6v = act6f.rearrange("p mh i (y x two) -> p mh i y x two",
                                  y=8, two=2)
            pl3 = apool.tile([128, 2, 32, 8, 4], FP16, name="pl3", tag="scr16")
            p3v = pl3.rearrange("p mh i (y two) x -> p mh i y two x", two=2)
            act6p = apool.tile([128, 2, 16, 32], FP16, name="act6p", tag="cparscr")
            a6pv = act6p.rearrange("p mh (y x) i -> p mh i y x", y=4)
            ps7 = [ppool.tile([128, 32], F32, name=f"ps7_{mh}", tag="ps", bufs=8)
                   for mh in range(4)]
            fc7e = ext["fc7s"]
            for h in range(2):
                s6, t6 = bn_affine_params(gs6[:, 2 * h:2 * h + 1],
                                          gs6[:, 2 * h + 1:2 * h + 2],
                                          sb["g6v"][:, h:h + 1],
                                          sb["be6v"][:, h:h + 1],
                                          N56, 128, f"L6h{h}",
                                          mask=mask[:, h:h + 1])
                nc.scalar.activation(a6fv[:, h], y6v[:, h], AF.Relu,
                                     bias=t6, scale=s6)
                nc.vector.tensor_tensor(pl3[:, h],
                                        a6v[:, h, :, :, :, 0:1].squeeze(4),
                                        a6v[:, h, :, :, :, 1:2].squeeze(4),
                                        ALU.max)
                nc.vector.tensor_tensor(a6pv[:, h],
                                        p3v[:, h, :, :, 0:1, :].squeeze(3),
                                        p3v[:, h, :, :, 1:2, :].squeeze(3),
                                        ALU.max)
                for pix in range(16):
                    kc = h * 16 + pix
                    w7 = spool.tile([128, 512], FP16, name="w7", tag="w7",
                                    bufs=3)
                    nc.sync.dma_start(w7[:], fc7e[:, kc, :])
                    rhs7 = act6p[:, h, pix, :]
                    for mh in range(4):
                        nc.tensor.matmul(ps7[mh][:],
                                         w7[:, mh * 128:(mh + 1) * 128],
                                         rhs7, start=(kc == 0), stop=(kc == 31))
            dbg("act6p", act6p[:])
            y7l = spool.tile([128, 4, 32], FP16, name="y7l")
            for mh in range(4):
                nc.scalar.copy(y7l[:, mh, :], ps7[mh][:])
            y7b = dpool.tile([512, 32], FP16, name="y7b")
            y7bv = y7b.rearrange("(mh p) b -> p mh b", mh=4)
            nc.sync.dma_start(y7bv[:], y7l[:])
            y7g = dpool.tile([4096, 32], FP16, name="y7g", addr_space="Shared")
            nc.gpsimd.collective_compute(
                "AllGather", ALU.bypass, replica_groups=REPLICA,
                ins=[y7b.opt()], outs=[y7g.opt()])
            y7gv = y7g.rearrange("(c mh p) b -> mh p c b", c=8, mh=4)
            act7 = apool.tile([128, 4, 256], FP16, name="act7", tag="cparscr")
            scr7 = spool.tile([128, 256], FP16, name="scr7", tag="fcscr")
            y7sb = spool.tile([128, 4, 8, 32], FP16, name="y7sb")
            ss7 = spool.tile([128, 2, 4], F32, name="ss7")
            for mh in range(4):
                nc.sync.dma_start(y7sb[:, mh], y7gv[mh])
                yv = y7sb[:, mh].rearrange("p a b -> p (a b)")
                nc.vector.tensor_scalar(scr7[:], yv, 0.0, 0.0, ALU.add,
                                        ALU.add, accum_out=ss7[:, 0, mh:mh + 1])
                nc.scalar.activation(scr7[:], yv, AF.Square,
                                     accum_out=ss7[:, 1, mh:mh + 1])
            s7, t7 = bn_affine_params(ss7[:, 0, :], ss7[:, 1, :],
                                      sb["g7v"][:], sb["be7v"][:],
                                      256, 128, "fc7", k=4)
            for mh in range(4):
                yv = y7sb[:, mh].rearrange("p a b -> p (a b)")
                nc.scalar.activation(act7[:, mh, :], yv, AF.Relu,
                                     bias=t7[:, mh:mh + 1],
                                     scale=s7[:, mh:mh + 1])
            dbg("act7", act7[:])

            # ================= FC8 =================
            ps8 = [ppool.tile([128, 256], F32, name=f"ps8_{mh}", tag="ps", bufs=8)
                   for mh in range(4)]
            for kc in range(4):
                for mh in range(4):
                    nc.tensor.matmul(ps8[mh][:],
                                     sb["fc8s"][:, kc, mh * 128:(mh + 1) * 128],
                                     act7[:, kc, :],
                                     start=(kc == 0), stop=(kc == 3))
            act8 = apool.tile([128, 4, 256], FP16, name="act8", tag="cparscr")
            ss8 = spool.tile([128, 2, 4], F32, name="ss8")
            for mh in range(4):
                nc.vector.tensor_scalar(scr7[:], ps8[mh][:], 0.0, 0.0, ALU.add,
                                        ALU.add, accum_out=ss8[:, 0, mh:mh + 1])
                nc.scalar.activation(scr7[:], ps8[mh][:], AF.Square,
                                     accum_out=ss8[:, 1, mh:mh + 1])
            s8, t8 = bn_affine_params(ss8[:, 0, :], ss8[:, 1, :],
                                      sb["g8v"][:], sb["be8v"][:],
                                      256, 128, "fc8", k=4)
            for mh in range(4):
                nc.scalar.activation(act8[:, mh, :], ps8[mh][:], AF.Relu,
                                     bias=t8[:, mh:mh + 1],
                                     scale=s8[:, mh:mh + 1])
            dbg("act8", act8[:])

            # ================= FC9 =================
            ps9 = ppool.tile([10, 256], F32, name="ps9", tag="ps", bufs=8)
            for kc in range(4):
                nc.tensor.matmul(ps9[:], sb["fc9s"][:, kc, :], act8[:, kc, :],
                                 start=(kc == 0), stop=(kc == 3))
            out_sb = spool.tile([10, 256], F32, name="out_sb")
            nc.vector.tensor_scalar_add(out_sb[:], ps9[:], sb["fc9bv"][:])
            nc.sync.dma_start(out_ext[:].transpose([1, 0]), out_sb[:])

    nc.compile()
    return nc, dbg_ext


_CACHE = {}


def _get_nc(debug_taps=()):
    key = tuple(sorted(debug_taps))
    if key not in _CACHE:
        _CACHE[key] = build_nc(debug_taps)
    return _CACHE[key]


def kernel(_debug_taps=(), _trace=False, **inputs):
    _install_ntff_hook()
    x1cols, shared = _host_prep(inputs)
    nc, dbg_ext = _get_nc(_debug_taps)
    in_maps = []
    for core in range(N_CORES):
        m = {"x1col": x1cols[core]}
        m.update(shared)
        in_maps.append(m)
    res = run_bass_kernel_spmd(nc, in_maps, core_ids=list(range(N_CORES)),
                               trace=_trace)
    out = res.results[0]["out"]
    if _debug_taps or _trace:
        return out, res
    return out


if __name__ == "__main__":
    rng = np.random.RandomState(0)
    ins = {"x": rng.randn(256, 3, 32, 32).astype(np.float32)}
    shapes = [(64, 3), (64, 64), (128, 64), (128, 128), (256, 128), (256, 256)]
    for i, (co, ci) in enumerate(shapes, start=1):
        ins[f"w{i}"] = (rng.randn(co, ci, 3, 3) * 0.05).astype(np.float32)
        ins[f"b{i}"] = np.zeros(co, np.float32)
        ins[f"g{i}"] = np.ones(co, np.float32)
        ins[f"be{i}"] = np.zeros(co, np.float32)
    ins["hash_a"] = rng.randn(2, 2306).astype(np.float32)
    ins["fc7_w"] = (rng.randn(512, 4096) * 0.02).astype(np.float32)
    ins["fc7_b"] = np.zeros(512, np.float32)
    ins["g7"] = np.ones(512, np.float32)
    ins["be7"] = np.zeros(512, np.float32)
    ins["fc8_w"] = (rng.randn(512, 512) * 0.02).astype(np.float32)
    ins["fc8_b"] = np.zeros(512, np.float32)
    ins["g8"] = np.ones(512, np.float32)
    ins["be8"] = np.zeros(512, np.float32)
    ins["fc9_w"] = (rng.randn(10, 512) * 0.02).astype(np.float32)
    ins["fc9_b"] = np.zeros(10, np.float32)
    out = kernel(**ins)
    print("out", out.shape, out.dtype, np.abs(out).mean())


# revision 33
# speedup vs baseline: 1.1371x; 1.0248x over previous
"""Bass/Trainium2 kernel for nn_ALSHVGGNet (8 NeuronCores, data parallel).

Strategy:
- Batch 256 sharded 32/core; all conv/fc weights replicated (host-prepped fp16
  layouts); fp16 matmuls with f32 PSUM accumulation.
- BatchNorm uses full-batch statistics: per-layer per-channel (sum, sumsq)
  computed on-device via bn_stats and AllReduce'd across the 8 cores.
- Conv1/Conv2 (64 ch) run in a (parity, channel) packed layout: 2 images share
  the 128 partitions via block-diagonal weights so DVE/ACT epilogues use all
  128 lanes.
- Conv3 pairs taps along K (shifted activation copy on partitions 64..127).
- ALSH mask: filter codes precomputed on host from w6/hash_a (weights only);
  query code from the all-reduced act5 channel sums (sign-invariant
  simplification of the reference math); mask folded into BN6's affine.
- FC stack: act6 pooled activations AllGather'd, then every core computes the
  full-batch FC7/8/9 locally (BN7/8 stats become core-local).
"""

import os
import sys
import types

sys.path.insert(0, "/opt/trn_rl_repo")

import numpy as np

import concourse.bass as bass
import concourse.mybir as mybir
import concourse.tile as tile
from concourse import bacc
from concourse.bass_utils import run_bass_kernel_spmd

N_CORES = 8
SHARD = 32          # images per core
EPS = 1e-5
U = 0.999
F32 = mybir.dt.float32
FP16 = mybir.dt.float16
AX = mybir.AxisListType
ALU = mybir.AluOpType
AF = mybir.ActivationFunctionType

# global counts for BN stats normalization
N12 = 256 * 1024    # layers 1,2
N34 = 256 * 256     # layers 3,4
N56 = 256 * 64      # layers 5,6


def _install_ntff_hook():
    """Best effort registration of the axon NTFF profile hook (timing only)."""
    try:
        import antenv
        from trn_agent_boot.trn_boot import _ntff_profile_via_ctypes

        hooks = types.ModuleType("antenv.axon_hooks")
        hook = _ntff_profile_via_ctypes("/opt/axon/libaxon_pjrt.so")
        hooks.get_axon_ntff_profile_hook = lambda: hook
        hooks.set_axon_ntff_profile_hook = lambda h: None
        sys.modules["antenv.axon_hooks"] = hooks
        antenv.axon_hooks = hooks
    except Exception:
        pass


# ---------------------------------------------------------------------------
# Host-side input preparation
# ---------------------------------------------------------------------------

def _host_prep(inputs):
    """Build per-core and shared device input arrays from the raw inputs."""
    f16 = np.float16
    d = {}

    x = np.asarray(inputs["x"], np.float32)           # (256, 3, 32, 32)
    B = x.shape[0]
    assert B == N_CORES * SHARD

    # --- x im2col in (parity-block, tap, ci) x (pair, pix) layout ----------
    xp = np.zeros((B, 3, 34, 34), np.float32)
    xp[:, :, 1:33, 1:33] = x
    x1cols = []
    for core in range(N_CORES):
        sh = xp[core * SHARD:(core + 1) * SHARD]      # (32, 3, 34, 34)
        col = np.zeros((2, 9, 3, 16, 1024), np.float32)
        for dy in range(3):
            for dx in range(3):
                w = sh[:, :, dy:dy + 32, dx:dx + 32]  # (32, 3, 32, 32)
                w = w.reshape(16, 2, 3, 1024)
                col[:, dy * 3 + dx] = w.transpose(1, 2, 0, 3)
        x1cols.append(np.ascontiguousarray(
            col.reshape(54, 16 * 1024)).astype(f16))

    def w_tap(w):  # (O, I, 3, 3) -> [tap][I, O]
        return [np.ascontiguousarray(w[:, :, t // 3, t % 3].T) for t in range(9)]

    w1 = np.asarray(inputs["w1"], np.float32)
    w2 = np.asarray(inputs["w2"], np.float32)
    w3 = np.asarray(inputs["w3"], np.float32)
    w4 = np.asarray(inputs["w4"], np.float32)
    w5 = np.asarray(inputs["w5"], np.float32)
    w6 = np.asarray(inputs["w6"], np.float32)

    # L1 block-diag [54, 128]
    w1bd = np.zeros((54, 128), np.float32)
    for t in range(9):
        blk = w1[:, :, t // 3, t % 3].T               # (3 ci, 64 co)
        for par in range(2):
            w1bd[par * 27 + t * 3:par * 27 + t * 3 + 3, par * 64:par * 64 + 64] = blk
    d["w1bd"] = w1bd.astype(f16)

    # L2 block-diag per tap [128, 9, 128]
    w2t = w_tap(w2)
    w2bd = np.zeros((128, 9, 128), np.float32)
    for t in range(9):
        for par in range(2):
            w2bd[par * 64:par * 64 + 64, t, par * 64:par * 64 + 64] = w2t[t]
    d["w2bd"] = w2bd.astype(f16)

    # L3 tap-paired passes [128, 6, 128]: rows (s*64+ci)
    w3t = w_tap(w3)
    w3p = np.zeros((128, 6, 128), np.float32)
    for dy in range(3):
        w3p[0:64, 2 * dy, :] = w3t[dy * 3 + 0]
        w3p[64:128, 2 * dy, :] = w3t[dy * 3 + 1]
        w3p[0:64, 2 * dy + 1, :] = w3t[dy * 3 + 2]
    d["w3p"] = w3p.astype(f16)

    d["w4s"] = np.stack(w_tap(w4), axis=1).astype(f16)          # [128, 9, 128]
    d["w5s"] = np.stack(w_tap(w5), axis=1).astype(f16)          # [128, 9, 256]
    w6s = np.zeros((128, 9, 2, 256), np.float32)
    for t in range(9):
        wt = w6[:, :, t // 3, t % 3].T                           # (256 ci, 256 co)
        w6s[:, t, 0, :] = wt[0:128]
        w6s[:, t, 1, :] = wt[128:256]
    d["w6s"] = w6s.astype(f16)

    fc7 = np.asarray(inputs["fc7_w"], np.float32)                # (512, 4096)
    d["fc7s"] = np.ascontiguousarray(
        fc7.reshape(512, 2, 128, 16).transpose(2, 1, 3, 0)
        .reshape(128, 32, 512)).astype(f16)
    fc8 = np.asarray(inputs["fc8_w"], np.float32)                # (512, 512)
    d["fc8s"] = np.ascontiguousarray(
        fc8.T.reshape(4, 128, 512).transpose(1, 0, 2)).astype(f16)
    fc9 = np.asarray(inputs["fc9_w"], np.float32)                # (10, 512)
    d["fc9s"] = np.ascontiguousarray(
        fc9.T.reshape(4, 128, 10).transpose(1, 0, 2)).astype(f16)
    d["fc9bv"] = np.asarray(inputs["fc9_b"], np.float32).reshape(10, 1)

    # gamma/beta in stat layouts (f32)
    for i, shape in [(1, (64, 1)), (2, (64, 1)), (3, (128, 1)), (4, (128, 1))]:
        d[f"g{i}v"] = np.asarray(inputs[f"g{i}"], np.float32).reshape(shape)
        d[f"be{i}v"] = np.asarray(inputs[f"be{i}"], np.float32).reshape(shape)
    for i in (5, 6):
        d[f"g{i}v"] = np.ascontiguousarray(
            np.asarray(inputs[f"g{i}"], np.float32).reshape(2, 128).T)
        d[f"be{i}v"] = np.ascontiguousarray(
            np.asarray(inputs[f"be{i}"], np.float32).reshape(2, 128).T)
    for i in (7, 8):
        d[f"g{i}v"] = np.ascontiguousarray(
            np.asarray(inputs[f"g{i}"], np.float32).reshape(4, 128).T)
        d[f"be{i}v"] = np.ascontiguousarray(
            np.asarray(inputs[f"be{i}"], np.float32).reshape(4, 128).T)

    # fold/broadcast helpers for the (parity, channel) layers
    fold = np.zeros((128, 64), np.float32)
    bc = np.zeros((64, 128), np.float32)
    for c in range(64):
        fold[c, c] = fold[64 + c, c] = 1.0
        bc[c, c] = bc[c, 64 + c] = 1.0
    d["fold64"] = fold
    d["bc64"] = bc
    d["ones2"] = np.ones((2, 128), np.float32)
    d["id2"] = np.eye(2, dtype=np.float32)

    # --- ALSH host precompute (weights only) -------------------------------
    hash_a = np.asarray(inputs["hash_a"], np.float32)            # (2, 2306)
    wf = w6.reshape(256, -1)
    norms = np.linalg.norm(wf, axis=1)
    wf_s = wf * (U / norms.max())
    ns = np.linalg.norm(wf_s, axis=1)
    P = np.concatenate([wf_s, (ns ** 2)[:, None], (ns ** 4)[:, None]], axis=1)
    bits_f = (P @ hash_a.T) > 0                                   # (256, 2)
    f0 = bits_f[:, 0].astype(np.float32).reshape(2, 128).T        # [128, 2(mh)]
    f1 = bits_f[:, 1].astype(np.float32).reshape(2, 128).T
    d["f0b"] = np.ascontiguousarray(f0)
    d["f1b"] = np.ascontiguousarray(f1)
    ha9 = hash_a[:, :2304].reshape(2, 9, 256).sum(1)              # (2 bits, 256 c)
    ha9m = np.zeros((128, 2, 2), np.float32)                      # (p, mh, j)
    for mh in range(2):
        ha9m[:, mh, :] = ha9[:, mh * 128:(mh + 1) * 128].T
    d["ha9m"] = ha9m

    shared = d
    return x1cols, shared


SHARED_SPECS = {
    "w1bd": ((54, 128), FP16), "w2bd": ((128, 9, 128), FP16),
    "w3p": ((128, 6, 128), FP16), "w4s": ((128, 9, 128), FP16),
    "w5s": ((128, 9, 256), FP16), "w6s": ((128, 9, 2, 256), FP16),
    "fc7s": ((128, 32, 512), FP16), "fc8s": ((128, 4, 512), FP16),
    "fc9s": ((128, 4, 10), FP16), "fc9bv": ((10, 1), F32),
    "g1v": ((64, 1), F32), "be1v": ((64, 1), F32),
    "g2v": ((64, 1), F32), "be2v": ((64, 1), F32),
    "g3v": ((128, 1), F32), "be3v": ((128, 1), F32),
    "g4v": ((128, 1), F32), "be4v": ((128, 1), F32),
    "g5v": ((128, 2), F32), "be5v": ((128, 2), F32),
    "g6v": ((128, 2), F32), "be6v": ((128, 2), F32),
    "g7v": ((128, 4), F32), "be7v": ((128, 4), F32),
    "g8v": ((128, 4), F32), "be8v": ((128, 4), F32),
    "fold64": ((128, 64), F32), "bc64": ((64, 128), F32),
    "ones2": ((2, 128), F32), "id2": ((2, 2), F32),
    "f0b": ((128, 2), F32), "f1b": ((128, 2), F32),
    "ha9m": ((128, 2, 2), F32),
}

REPLICA = [list(range(N_CORES))]


def build_nc(debug_taps=()):
    nc = bacc.Bacc("TRN2", target_bir_lowering=False, debug=False,
                   num_devices=N_CORES)

    x1col_ext = nc.dram_tensor("x1col", [54, 16384], FP16, kind="ExternalInput")
    ext = {}
    for name, (shape, dt) in SHARED_SPECS.items():
        ext[name] = nc.dram_tensor(name, list(shape), dt, kind="ExternalInput")
    out_ext = nc.dram_tensor("out", [256, 10], F32, kind="ExternalOutput")
    dbg_ext = {}

    with tile.TileContext(nc) as tc:
        with (
            tc.tile_pool(name="const", bufs=1) as cpool,
            tc.tile_pool(name="acts", bufs=1) as apool,
            tc.tile_pool(name="scr", bufs=1) as spool,
            tc.tile_pool(name="psum", bufs=1, space="PSUM") as ppool,
            tc.tile_pool(name="dram", bufs=1, space="DRAM") as dpool,
        ):
            # ---- persistent consts/weights in SBUF (fc7s streamed later) ----
            sb = {}
            for name, (shape, dt) in SHARED_SPECS.items():
                if name == "fc7s":
                    continue
                t = cpool.tile(list(shape), dt, name=f"sb_{name}")
                nc.sync.dma_start(t[:], ext[name][:])
                sb[name] = t

            def dbg(name, ap):
                if name in debug_taps:
                    sh = [int(s) for s in ap.shape]
                    dt = ap.dtype
                    dbg_ext[name] = nc.dram_tensor(f"dbg_{name}", sh, dt,
                                                   kind="ExternalOutput")
                    nc.sync.dma_start(dbg_ext[name][:], ap)

            # ---- small helper chains ----
            def stat_combine(statv, T, half_cnt, name):
                """statv: [128, T, 6] bn_stats rows -> sums [128, 2] (sum, sumsq).

                half_cnt = per-tile even/odd element count (FD/2).
                """
                sm = spool.tile([128, 4], F32, name=f"sm_{name}")
                # sum of means (even + odd)
                nc.vector.tensor_reduce(sm[:, 0:1], statv[:, :, 1:2].squeeze(2),
                                        AX.X, ALU.add)
                nc.vector.tensor_reduce(sm[:, 1:2], statv[:, :, 4:5].squeeze(2),
                                        AX.X, ALU.add)
                # sum of count*var
                nc.vector.tensor_reduce(sm[:, 2:3], statv[:, :, 2:3].squeeze(2),
                                        AX.X, ALU.add)
                nc.vector.tensor_reduce(sm[:, 3:4], statv[:, :, 5:6].squeeze(2),
                                        AX.X, ALU.add)
                # sum of means^2
                msq = spool.tile([128, 2 * T], F32, name=f"msq_{name}")
                nc.vector.tensor_tensor(msq[:, 0:T], statv[:, :, 1:2].squeeze(2),
                                        statv[:, :, 1:2].squeeze(2), ALU.mult)
                nc.vector.tensor_tensor(msq[:, T:2 * T], statv[:, :, 4:5].squeeze(2),
                                        statv[:, :, 4:5].squeeze(2), ALU.mult)
                m2 = spool.tile([128, 1], F32, name=f"m2_{name}")
                nc.vector.tensor_reduce(m2[:], msq[:], AX.X, ALU.add)

                sums = spool.tile([128, 2], F32, name=f"sums_{name}")
                # sum = half_cnt * (sm0 + sm1)
                t0 = spool.tile([128, 1], F32, name=f"t0_{name}")
                nc.vector.tensor_tensor(t0[:], sm[:, 0:1], sm[:, 1:2], ALU.add)
                nc.vector.tensor_scalar_mul(sums[:, 0:1], t0[:], float(half_cnt))
                # sumsq = sm2 + sm3 + half_cnt * m2
                t1 = spool.tile([128, 1], F32, name=f"t1_{name}")
                nc.vector.tensor_tensor(t1[:], sm[:, 2:3], sm[:, 3:4], ALU.add)
                nc.vector.tensor_scalar(sums[:, 1:2], m2[:], float(half_cnt),
                                        None, ALU.mult)
                nc.vector.tensor_tensor(sums[:, 1:2], sums[:, 1:2], t1[:], ALU.add)
                return sums

            def allreduce(sums_ap, cols, name):
                ib = dpool.tile([128, cols], F32, name=f"arin_{name}")
                ob = dpool.tile([128, cols], F32, name=f"arout_{name}")
                nc.sync.dma_start(ib[:], sums_ap)
                nc.gpsimd.collective_compute(
                    "AllReduce", ALU.add, replica_groups=REPLICA,
                    ins=[ib.opt()], outs=[ob.opt()])
                g = spool.tile([128, cols], F32, name=f"gsum_{name}")
                nc.sync.dma_start(g[:], ob[:])
                return g

            def bn_affine_params(S, Q, g_ap, be_ap, n_total, P_, name,
                                 mask=None, k=1):
                """S,Q: [P_,k] global sum/sumsq -> (s, t) [P_,k] f32 tiles."""
                st = spool.tile([P_, 8 * k], F32, name=f"st_{name}")
                m, v, w, r0, a, dtmp, s_t, t_t = [st[:, i * k:(i + 1) * k]
                                                  for i in range(8)]
                nc.vector.tensor_scalar_mul(m, S, 1.0 / n_total)
                msq = spool.tile([P_, k], F32, name=f"stm_{name}")
                nc.vector.tensor_tensor(msq, m, m, ALU.mult)
                nc.vector.tensor_scalar_mul(v, Q, 1.0 / n_total)
                nc.vector.tensor_tensor(v, v, msq, ALU.subtract)
                if mask is not None:
                    nc.vector.tensor_tensor(m, m, mask, ALU.mult)
                    nc.vector.tensor_tensor(v, v, mask, ALU.mult)
                nc.vector.tensor_scalar_add(v, v, EPS)   # v := var + eps
                nc.vector.reciprocal(w, v)
                nc.scalar.activation(r0, w, AF.Sqrt)
                # Newton step: r1 = r0 * (1.5 - 0.5 * v * r0^2)
                nc.vector.tensor_tensor(a, r0, r0, ALU.mult)
                nc.vector.tensor_tensor(a, a, v, ALU.mult)
                nc.vector.tensor_scalar(a, a, -0.5, 1.5, ALU.mult, ALU.add)
                nc.vector.tensor_tensor(r0, r0, a, ALU.mult)
                nc.vector.tensor_tensor(s_t, g_ap, r0, ALU.mult)
                if mask is not None:
                    nc.vector.tensor_tensor(s_t, s_t, mask, ALU.mult)
                nc.vector.tensor_tensor(dtmp, m, s_t, ALU.mult)
                nc.vector.tensor_tensor(t_t, be_ap, dtmp, ALU.subtract)
                return s_t, t_t

            def fold_bcast(sums, g_ap, be_ap, n_total, name):
                """(parity, channel) stats: fold to 64, bn math, broadcast to 128."""
                up = spool.tile([64, 2], F32, name=f"up_{name}")
                nc.vector.tensor_copy(up[:], sums[64:128, :])
                s64 = spool.tile([64, 2], F32, name=f"s64_{name}")
                nc.vector.tensor_tensor(s64[:], sums[0:64, :], up[:], ALU.add)
                s_t, t_t = bn_affine_params(s64[:, 0:1], s64[:, 1:2],
                                            g_ap, be_ap, n_total, 64, name)
                st128 = spool.tile([128, 2], F32, name=f"stb_{name}")
                nc.vector.tensor_copy(st128[0:64, 0:1], s_t)
                nc.vector.tensor_copy(st128[0:64, 1:2], t_t)
                nc.vector.tensor_copy(st128[64:128, :], st128[0:64, :])
                return st128[:, 0:1], st128[:, 1:2]

            # ================= Layer 1 =================
            y1 = apool.tile([128, 16384], FP16, name="y1", tag="ybuf")
            stat1 = spool.tile([128, 32, 6], F32, name="stat1")
            for t in range(32):
                if t % 8 == 0:
                    x1t = spool.tile([54, 8, 512], FP16, name="x1t", tag="x1t",
                                     bufs=3)
                    nc.gpsimd.dma_start(
                        x1t[:], x1col_ext[:, t * 512:(t + 8) * 512]
                        .rearrange("p (a b) -> p a b", a=8))
                ps = ppool.tile([128, 512], F32, name=f"ps1_{t}", tag="ps", bufs=8)
                nc.tensor.matmul(ps[:], sb["w1bd"][:], x1t[:, t % 8, :],
                                 start=True, stop=True)
                nc.vector.bn_stats(stat1[:, t, :], ps[:])
                nc.scalar.copy(y1[:, t * 512:(t + 1) * 512], ps[:])
            dbg("y1", y1[:])
            sums1 = stat_combine(stat1, 32, 256, "L1")
            gs1 = allreduce(sums1[:], 2, "L1")
            s1, t1 = fold_bcast(gs1, sb["g1v"][:], sb["be1v"][:], N12, "L1")

            act1 = apool.tile([128, 16, 34, 34], FP16, name="act1", tag="actpad")
            nc.gpsimd.memset(act1[:, :, 0:1, :], 0.0)
            nc.gpsimd.memset(act1[:, :, 33:34, :], 0.0)
            nc.gpsimd.memset(act1[:, :, 1:33, 0:1], 0.0)
            nc.gpsimd.memset(act1[:, :, 1:33, 33:34], 0.0)
            y1v = y1.rearrange("p (pr y x) -> p pr y x", pr=16, y=32, x=32)
            for p0, pn in [(0, 1), (1, 3), (4, 4), (8, 4), (12, 4)]:
                nc.scalar.activation(act1[:, p0:p0 + pn, 1:33, 1:33],
                                     y1v[:, p0:p0 + pn], AF.Relu,
                                     bias=t1, scale=s1)
            dbg("act1", act1[:])

            # ================= Layer 2 =================
            y2 = apool.tile([128, 16384], FP16, name="y2", tag="ybuf")
            stat2 = spool.tile([128, 32, 6], F32, name="stat2")
            for prg in range(8):
                pss = [ppool.tile([128, 512], F32, name=f"ps2_{prg}_{u}",
                                  tag="ps", bufs=8) for u in range(4)]
                for t in range(9):
                    dy, dx = t // 3, t % 3
                    for u in range(4):
                        pr, h = prg * 2 + u // 2, u % 2
                        rhs = act1[:, pr, h * 16 + dy:h * 16 + dy + 16,
                                   dx:dx + 32]
                        nc.tensor.matmul(pss[u][:], sb["w2bd"][:, t, :], rhs,
                                         start=(t == 0), stop=(t == 8))
                for u in range(4):
                    ti = prg * 4 + u
                    nc.vector.bn_stats(stat2[:, ti, :], pss[u][:])
                    nc.scalar.copy(y2[:, ti * 512:(ti + 1) * 512], pss[u][:])
            dbg("y2", y2[:])
            sums2 = stat_combine(stat2, 32, 256, "L2")
            gs2 = allreduce(sums2[:], 2, "L2")
            s2, t2 = fold_bcast(gs2, sb["g2v"][:], sb["be2v"][:], N12, "L2")

            act2f = apool.tile([128, 16, 1024], FP16, name="act2f", tag="actfull")
            y2v = y2.rearrange("p (pr q) -> p pr q", pr=16)
            a2v = act2f.rearrange("p pr (y x two) -> p pr y x two", y=32, two=2)
            pl1 = apool.tile([128, 16, 32, 16], FP16, name="pl1", tag="scr16")
            p1v = pl1.rearrange("p pr (y two) x -> p pr y two x", two=2)
            cpar = apool.tile([128, 16, 256], FP16, name="cpar", tag="cparscr")
            cpv = cpar.rearrange("p pr (y x) -> p pr y x", y=16)

            # scatter into act2p [128=(s,c), 32 img, 18, 18] with shift copy
            act2p = apool.tile([128, 32, 18, 18], FP16, name="act2p", tag="actp")
            nc.gpsimd.memset(act2p[:, :, 0:1, :], 0.0)
            nc.gpsimd.memset(act2p[:, :, 17:18, :], 0.0)
            nc.gpsimd.memset(act2p[0:64, :, 1:17, 0:1], 0.0)
            nc.gpsimd.memset(act2p[0:64, :, 1:17, 17:18], 0.0)
            nc.gpsimd.memset(act2p[64:128, :, 1:17, 16:18], 0.0)
            a2pv = act2p.rearrange("p (i ip) y x -> p i ip y x", ip=2)
            cp4 = cpar.rearrange("p pr (y x) -> p pr y x", y=16)
            for p0 in range(0, 16, 4):
                sl = slice(p0, p0 + 4)
                nc.scalar.activation(act2f[:, sl, :], y2v[:, sl],
                                     AF.Relu, bias=t2, scale=s2)
                nc.vector.tensor_tensor(pl1[:, sl],
                                        a2v[:, sl, :, :, 0:1].squeeze(4),
                                        a2v[:, sl, :, :, 1:2].squeeze(4),
                                        ALU.max)
                nc.vector.tensor_tensor(cpv[:, sl],
                                        p1v[:, sl, :, 0:1, :].squeeze(3),
                                        p1v[:, sl, :, 1:2, :].squeeze(3),
                                        ALU.max)
                nc.vector.tensor_copy(a2pv[0:64, sl, 0, 1:17, 1:17],
                                      cp4[0:64, sl])
                nc.gpsimd.tensor_copy(a2pv[64:128, sl, 1, 1:17, 0:16],
                                      cp4[64:128, sl])
                for pr in range(p0, p0 + 4):
                    nc.gpsimd.dma_start(a2pv[0:64, pr, 1, 1:17, 1:17],
                                        cp4[64:128, pr])
                    nc.gpsimd.dma_start(a2pv[64:128, pr, 0, 1:17, 0:16],
                                        cp4[0:64, pr])
            dbg("act2p", act2p[:])

            # ================= Layer 3 =================
            y3 = apool.tile([128, 32, 256], FP16, name="y3", tag="ybuf")
            stat3 = spool.tile([128, 16, 6], F32, name="stat3")
            passes = [(0, 0), (0, 2), (1, 0), (1, 2), (2, 0), (2, 2)]
            for tg in range(8):
                pss = [ppool.tile([128, 512], F32, name=f"ps3_{tg}_{u}",
                                  tag="ps", bufs=8) for u in range(2)]
                for pi, (dy, dx) in enumerate(passes):
                    for u in range(2):
                        i0 = (tg * 2 + u) * 2
                        rhs = act2p[:, i0:i0 + 2, dy:dy + 16, dx:dx + 16]
                        nc.tensor.matmul(pss[u][:],
                                         sb["w3p"][:, 2 * dy + (dx // 2), :],
                                         rhs, start=(pi == 0), stop=(pi == 5))
                for u in range(2):
                    ti = tg * 2 + u
                    i0 = ti * 2
                    nc.vector.bn_stats(stat3[:, ti, :], pss[u][:])
                    nc.scalar.copy(
                        y3[:, i0:i0 + 2, :].rearrange("p a b -> p (a b)"),
                        pss[u][:])
            dbg("y3", y3[:])
            sums3 = stat_combine(stat3, 16, 256, "L3")
            gs3 = allreduce(sums3[:], 2, "L3")
            s3, t3 = bn_affine_params(gs3[:, 0:1], gs3[:, 1:2], sb["g3v"][:],
                                      sb["be3v"][:], N34, 128, "L3")

            act3 = apool.tile([128, 32, 18, 18], FP16, name="act3", tag="actpad")
            nc.gpsimd.memset(act3[:, :, 0:1, :], 0.0)
            nc.gpsimd.memset(act3[:, :, 17:18, :], 0.0)
            nc.gpsimd.memset(act3[:, :, 1:17, 0:1], 0.0)
            nc.gpsimd.memset(act3[:, :, 1:17, 17:18], 0.0)
            y3v = y3.rearrange("p i (y x) -> p i y x", y=16)
            for i0, ni in [(0, 2), (2, 6), (8, 8), (16, 8), (24, 8)]:
                nc.scalar.activation(act3[:, i0:i0 + ni, 1:17, 1:17],
                                     y3v[:, i0:i0 + ni], AF.Relu,
                                     bias=t3, scale=s3)
            dbg("act3", act3[:])

            # ================= Layer 4 =================
            y4 = apool.tile([128, 32, 256], FP16, name="y4", tag="ybuf")
            stat4 = spool.tile([128, 16, 6], F32, name="stat4")
            for tg in range(4):
                pss = [ppool.tile([128, 512], F32, name=f"ps4_{tg}_{u}",
                                  tag="ps", bufs=8) for u in range(4)]
                for t in range(9):
                    dy, dx = t // 3, t % 3
                    for u in range(4):
                        i0 = (tg * 4 + u) * 2
                        rhs = act3[:, i0:i0 + 2, dy:dy + 16, dx:dx + 16]
                        nc.tensor.matmul(pss[u][:], sb["w4s"][:, t, :], rhs,
                                         start=(t == 0), stop=(t == 8))
                for u in range(4):
                    ti = tg * 4 + u
                    i0 = ti * 2
                    nc.vector.bn_stats(stat4[:, ti, :], pss[u][:])
                    nc.scalar.copy(
                        y4[:, i0:i0 + 2, :].rearrange("p a b -> p (a b)"),
                        pss[u][:])
            dbg("y4", y4[:])
            sums4 = stat_combine(stat4, 16, 256, "L4")
            gs4 = allreduce(sums4[:], 2, "L4")
            s4, t4 = bn_affine_params(gs4[:, 0:1], gs4[:, 1:2], sb["g4v"][:],
                                      sb["be4v"][:], N34, 128, "L4")

            act4f = apool.tile([128, 32, 256], FP16, name="act4f", tag="actfull")
            y4v = y4.rearrange("p i (y x) -> p i y x", y=16)
            a4fv = act4f.rearrange("p i (y x) -> p i y x", y=16)
            a4v = act4f.rearrange("p i (y x two) -> p i y x two", y=16, two=2)
            pl2 = apool.tile([128, 32, 16, 8], FP16, name="pl2", tag="scr16")
            p2v = pl2.rearrange("p i (y two) x -> p i y two x", two=2)
            for i0 in range(0, 32, 8):
                sl = slice(i0, i0 + 8)
                nc.scalar.activation(a4fv[:, sl], y4v[:, sl],
                                     AF.Relu, bias=t4, scale=s4)
                nc.vector.tensor_tensor(pl2[:, sl],
                                        a4v[:, sl, :, :, 0:1].squeeze(4),
                                        a4v[:, sl, :, :, 1:2].squeeze(4),
                                        ALU.max)
            act4p = apool.tile([128, 32, 10, 10], FP16, name="act4p", tag="actp")
            nc.gpsimd.memset(act4p[:, :, 0:1, :], 0.0)
            nc.gpsimd.memset(act4p[:, :, 9:10, :], 0.0)
            nc.gpsimd.memset(act4p[:, :, 1:9, 0:1], 0.0)
            nc.gpsimd.memset(act4p[:, :, 1:9, 9:10], 0.0)
            for i0 in range(0, 32, 8):
                sl = slice(i0, i0 + 8)
                nc.vector.tensor_tensor(act4p[:, sl, 1:9, 1:9],
                                        p2v[:, sl, :, 0:1, :].squeeze(3),
                                        p2v[:, sl, :, 1:2, :].squeeze(3),
                                        ALU.max)
            dbg("act4p", act4p[:])

            # ================= Layer 5 =================
            y5 = apool.tile([128, 2, 32, 64], FP16, name="y5", tag="ybuf")
            stat5 = spool.tile([128, 2, 4, 6], F32, name="stat5")
            for mh in range(2):
                for tg in range(2):
                    pss = [ppool.tile([128, 512], F32, name=f"ps5_{mh}_{tg}_{u}",
                                      tag="ps", bufs=8) for u in range(2)]
                    for t in range(9):
                        dy, dx = t // 3, t % 3
                        for u in range(2):
                            i0 = (tg * 2 + u) * 8
                            rhs = act4p[:, i0:i0 + 8, dy:dy + 8, dx:dx + 8]
                            nc.tensor.matmul(
                                pss[u][:], sb["w5s"][:, t, mh * 128:(mh + 1) * 128],
                                rhs, start=(t == 0), stop=(t == 8))
                    for u in range(2):
                        ti = tg * 2 + u
                        i0 = ti * 8
                        nc.vector.bn_stats(stat5[:, mh, ti, :], pss[u][:])
                        nc.scalar.copy(
                            y5[:, mh, i0:i0 + 8, :].rearrange("p a b -> p (a b)"),
                            pss[u][:])
            dbg("y5", y5[:])
            sums5 = spool.tile([128, 4], F32, name="sums5")
            for h in range(2):
                sums5h = stat_combine(stat5[:, h], 4, 256, f"L5h{h}")
                nc.vector.tensor_copy(sums5[:, 2 * h:2 * h + 2], sums5h[:])
            gs5 = allreduce(sums5[:], 4, "L5")

            act5 = apool.tile([128, 2, 32, 10, 10], FP16, name="act5",
                              tag="actpad")
            for h in range(2):
                nc.gpsimd.memset(act5[:, h, :, 0:1, :], 0.0)
                nc.gpsimd.memset(act5[:, h, :, 9:10, :], 0.0)
                nc.gpsimd.memset(act5[:, h, :, 1:9, 0:1], 0.0)
                nc.gpsimd.memset(act5[:, h, :, 1:9, 9:10], 0.0)
            qsum = spool.tile([128, 2], F32, name="qsum")
            y5v = y5.rearrange("p mh i (y x) -> p mh i y x", y=8)
            for h in range(2):
                s5, t5 = bn_affine_params(gs5[:, 2 * h:2 * h + 1],
                                          gs5[:, 2 * h + 1:2 * h + 2],
                                          sb["g5v"][:, h:h + 1],
                                          sb["be5v"][:, h:h + 1],
                                          N56, 128, f"L5h{h}")
                nc.scalar.activation(act5[:, h, :, 1:9, 1:9], y5v[:, h],
                                     AF.Relu, bias=t5, scale=s5,
                                     accum_out=qsum[:, h:h + 1])
            dbg("act5", act5[:])

            # ================= Layer 6 =================
            # All 8 psum tiles open; K-chunk h=0 matmuls run for every tile
            # first, so the h=1 chunk (which needs act5 h1 / AR5b) comes last.
            y6 = apool.tile([128, 2, 32, 64], FP16, name="y6", tag="scr16")
            stat6 = spool.tile([128, 2, 4, 6], F32, name="stat6")
            ps6 = {}
            for mh in range(2):
                for ti in range(4):
                    ps6[(mh, ti)] = ppool.tile([128, 512], F32,
                                               name=f"ps6_{mh}_{ti}",
                                               tag="ps", bufs=8)
            for h in range(2):
                for t in range(9):
                    dy, dx = t // 3, t % 3
                    for mh in range(2):
                        for ti in range(4):
                            i0 = ti * 8
                            rhs = act5[:, h, i0:i0 + 8, dy:dy + 8, dx:dx + 8]
                            nc.tensor.matmul(
                                ps6[(mh, ti)][:],
                                sb["w6s"][:, t, h, mh * 128:(mh + 1) * 128],
                                rhs, start=(h == 0 and t == 0),
                                stop=(h == 1 and t == 8))
            for mh in range(2):
                for ti in range(4):
                    i0 = ti * 8
                    nc.vector.bn_stats(stat6[:, mh, ti, :], ps6[(mh, ti)][:])
                    nc.scalar.copy(
                        y6[:, mh, i0:i0 + 8, :].rearrange("p a b -> p (a b)"),
                        ps6[(mh, ti)][:])
            dbg("y6", y6[:])
            sums6 = spool.tile([128, 6], F32, name="sums6")
            for h in range(2):
                sums6h = stat_combine(stat6[:, h], 4, 256, f"L6h{h}")
                nc.vector.tensor_copy(sums6[:, 2 * h:2 * h + 2], sums6h[:])
            nc.vector.tensor_copy(sums6[:, 4:6], qsum[:])
            gs6 = allreduce(sums6[:], 6, "L6")

            # ---- ALSH mask from global qsums ----
            psd = ppool.tile([2, 2], F32, name="psd", tag="ps", bufs=8)
            for mh in range(2):
                nc.tensor.matmul(psd[:, 0:1], sb["ha9m"][:, mh, :],
                                 gs6[:, 4 + mh:5 + mh],
                                 start=(mh == 0), stop=(mh == 1))
            dsb = spool.tile([2, 2], F32, name="dsb")
            nc.vector.tensor_copy(dsb[:, 0:1], psd[:, 0:1])
            bq = spool.tile([2, 2], F32, name="bq")
            nc.vector.tensor_scalar(bq[:, 0:1], dsb[:, 0:1], 0.0, None, ALU.is_gt)
            bqd = spool.tile([2, 2], F32, name="bqd")
            nc.vector.tensor_scalar_mul(bqd[:], sb["id2"][:], bq[:, 0:1])
            psb2 = ppool.tile([128, 2], F32, name="psb2", tag="ps", bufs=8)
            nc.tensor.matmul(psb2[:], sb["ones2"][:], bqd[:],
                             start=True, stop=True)
            bqb = spool.tile([128, 2], F32, name="bqb")
            nc.vector.tensor_copy(bqb[:], psb2[:])
            mask = spool.tile([128, 2], F32, name="mask")
            e0 = spool.tile([128, 2], F32, name="e0")
            e0s = spool.tile([128, 2], F32, name="e0s")
            nc.vector.tensor_scalar(e0[:], sb["f0b"][:], bqb[:, 0:1], None,
                                    ALU.subtract)
            nc.scalar.activation(e0s[:], e0[:], AF.Square)
            nc.vector.tensor_scalar(mask[:], e0s[:], -1.0, 1.0, ALU.mult, ALU.add)
            e1 = spool.tile([128, 2], F32, name="e1")
            e1s = spool.tile([128, 2], F32, name="e1s")
            nc.vector.tensor_scalar(e1[:], sb["f1b"][:], bqb[:, 1:2], None,
                                    ALU.subtract)
            nc.scalar.activation(e1s[:], e1[:], AF.Square)
            nc.vector.tensor_scalar(e1s[:], e1s[:], -1.0, 1.0, ALU.mult, ALU.add)
            nc.vector.tensor_tensor(mask[:], mask[:], e1s[:], ALU.mult)
            dbg("mask", mask[:])

            act6f = apool.tile([128, 2, 32, 64], FP16, name="act6f",
                               tag="actfull")
            y6v = y6.rearrange("p mh i (y x) -> p mh i y x", y=8)
            a6fv = act6f.rearrange("p mh i (y x) -> p mh i y x", y=8)
            a6v = act6f.rearrange("p mh i (y x two) -> p mh i y x two",
                                  y=8, two=2)
            pl3 = apool.tile([128, 2, 32, 8, 4], FP16, name="pl3", tag="scr16")
            p3v = pl3.rearrange("p mh i (y two) x -> p mh i y two x", two=2)
            act6p = apool.tile([128, 2, 16, 32], FP16, name="act6p", tag="cparscr")
            a6pv = act6p.rearrange("p mh (y x) i -> p mh i y x", y=4)
            ps7 = [ppool.tile([128, 32], F32, name=f"ps7_{mh}", tag="ps", bufs=8)
                   for mh in range(4)]
            fc7e = ext["fc7s"]
            for h in range(2):
                s6, t6 = bn_affine_params(gs6[:, 2 * h:2 * h + 1],
                                          gs6[:, 2 * h + 1:2 * h + 2],
                                          sb["g6v"][:, h:h + 1],
                                          sb["be6v"][:, h:h + 1],
                                          N56, 128, f"L6h{h}",
                                          mask=mask[:, h:h + 1])
                nc.scalar.activation(a6fv[:, h], y6v[:, h], AF.Relu,
                                     bias=t6, scale=s6)
                nc.vector.tensor_tensor(pl3[:, h],
                                        a6v[:, h, :, :, :, 0:1].squeeze(4),
                                        a6v[:, h, :, :, :, 1:2].squeeze(4),
                                        ALU.max)
                nc.vector.tensor_tensor(a6pv[:, h],
                                        p3v[:, h, :, :, 0:1, :].squeeze(3),
                                        p3v[:, h, :, :, 1:2, :].squeeze(3),
                                        ALU.max)
                for pix in range(16):
                    kc = h * 16 + pix
                    w7 = spool.tile([128, 512], FP16, name="w7", tag="w7",
                                    bufs=3)
                    nc.sync.dma_start(w7[:], fc7e[:, kc, :])
                    rhs7 = act6p[:, h, pix, :]
                    for mh in range(4):
                        nc.tensor.matmul(ps7[mh][:],
                                         w7[:, mh * 128:(mh + 1) * 128],
                                         rhs7, start=(kc == 0), stop=(kc == 31))
            dbg("act6p", act6p[:])
            y7l = spool.tile([128, 4, 32], FP16, name="y7l")
            for mh in range(4):
                nc.scalar.copy(y7l[:, mh, :], ps7[mh][:])
            y7b = dpool.tile([512, 32], FP16, name="y7b")
            y7bv = y7b.rearrange("(mh p) b -> p mh b", mh=4)
            nc.sync.dma_start(y7bv[:], y7l[:])
            y7g = dpool.tile([4096, 32], FP16, name="y7g", addr_space="Shared")
            nc.gpsimd.collective_compute(
                "AllGather", ALU.bypass, replica_groups=REPLICA,
                ins=[y7b.opt()], outs=[y7g.opt()])
            y7gv = y7g.rearrange("(c mh p) b -> mh p c b", c=8, mh=4)
            act7 = apool.tile([128, 4, 256], FP16, name="act7", tag="cparscr")
            scr7 = spool.tile([128, 256], FP16, name="scr7", tag="fcscr")
            y7sb = spool.tile([128, 4, 8, 32], FP16, name="y7sb")
            ss7 = spool.tile([128, 2, 4], F32, name="ss7")
            for mh in range(4):
                nc.sync.dma_start(y7sb[:, mh], y7gv[mh])
                yv = y7sb[:, mh].rearrange("p a b -> p (a b)")
                nc.vector.tensor_scalar(scr7[:], yv, 0.0, 0.0, ALU.add,
                                        ALU.add, accum_out=ss7[:, 0, mh:mh + 1])
                nc.scalar.activation(scr7[:], yv, AF.Square,
                                     accum_out=ss7[:, 1, mh:mh + 1])
            s7, t7 = bn_affine_params(ss7[:, 0, :], ss7[:, 1, :],
                                      sb["g7v"][:], sb["be7v"][:],
                                      256, 128, "fc7", k=4)
            for mh in range(4):
                yv = y7sb[:, mh].rearrange("p a b -> p (a b)")
                nc.scalar.activation(act7[:, mh, :], yv, AF.Relu,
                                     bias=t7[:, mh:mh + 1],
                                     scale=s7[:, mh:mh + 1])
            dbg("act7", act7[:])

            # ================= FC8 =================
            ps8 = [ppool.tile([128, 256], F32, name=f"ps8_{mh}", tag="ps", bufs=8)
                   for mh in range(4)]
            for kc in range(4):
                for mh in range(4):
                    nc.tensor.matmul(ps8[mh][:],
                                     sb["fc8s"][:, kc, mh * 128:(mh + 1) * 128],
                                     act7[:, kc, :],
                                     start=(kc == 0), stop=(kc == 3))
            act8 = apool.tile([128, 4, 256], FP16, name="act8", tag="cparscr")
            ss8 = spool.tile([128, 2, 4], F32, name="ss8")
            for mh in range(4):
                nc.vector.tensor_scalar(scr7[:], ps8[mh][:], 0.0, 0.0, ALU.add,
                                        ALU.add, accum_out=ss8[:, 0, mh:mh + 1])
                nc.scalar.activation(scr7[:], ps8[mh][:], AF.Square,
                                     accum_out=ss8[:, 1, mh:mh + 1])
            s8, t8 = bn_affine_params(ss8[:, 0, :], ss8[:, 1, :],
                                      sb["g8v"][:], sb["be8v"][:],
                                      256, 128, "fc8", k=4)
            for mh in range(4):
                nc.scalar.activation(act8[:, mh, :], ps8[mh][:], AF.Relu,
                                     bias=t8[:, mh:mh + 1],
                                     scale=s8[:, mh:mh + 1])
            dbg("act8", act8[:])

            # ================= FC9 =================
            ps9 = ppool.tile([10, 256], F32, name="ps9", tag="ps", bufs=8)
            for kc in range(4):
                nc.tensor.matmul(ps9[:], sb["fc9s"][:, kc, :], act8[:, kc, :],
                                 start=(kc == 0), stop=(kc == 3))
            out_sb = spool.tile([10, 256], F32, name="out_sb")
            nc.vector.tensor_scalar_add(out_sb[:], ps9[:], sb["fc9bv"][:])
            nc.sync.dma_start(out_ext[:].transpose([1, 0]), out_sb[:])

    nc.compile()
    return nc, dbg_ext


_CACHE = {}


def _get_nc(debug_taps=()):
    key = tuple(sorted(debug_taps))
    if key not in _CACHE:
        _CACHE[key] = build_nc(debug_taps)
    return _CACHE[key]


def kernel(_debug_taps=(), _trace=False, **inputs):
    _install_ntff_hook()
    x1cols, shared = _host_prep(inputs)
    nc, dbg_ext = _get_nc(_debug_taps)
    in_maps = []
    for core in range(N_CORES):
        m = {"x1col": x1cols[core]}
        m.update(shared)
        in_maps.append(m)
    res = run_bass_kernel_spmd(nc, in_maps, core_ids=list(range(N_CORES)),
                               trace=_trace)
    out = res.results[0]["out"]
    if _debug_taps or _trace:
        return out, res
    return out


if __name__ == "__main__":
    rng = np.random.RandomState(0)
    ins = {"x": rng.randn(256, 3, 32, 32).astype(np.float32)}
    shapes = [(64, 3), (64, 64), (128, 64), (128, 128), (256, 128), (256, 256)]
    for i, (co, ci) in enumerate(shapes, start=1):
        ins[f"w{i}"] = (rng.randn(co, ci, 3, 3) * 0.05).astype(np.float32)
        ins[f"b{i}"] = np.zeros(co, np.float32)
        ins[f"g{i}"] = np.ones(co, np.float32)
        ins[f"be{i}"] = np.zeros(co, np.float32)
    ins["hash_a"] = rng.randn(2, 2306).astype(np.float32)
    ins["fc7_w"] = (rng.randn(512, 4096) * 0.02).astype(np.float32)
    ins["fc7_b"] = np.zeros(512, np.float32)
    ins["g7"] = np.ones(512, np.float32)
    ins["be7"] = np.zeros(512, np.float32)
    ins["fc8_w"] = (rng.randn(512, 512) * 0.02).astype(np.float32)
    ins["fc8_b"] = np.zeros(512, np.float32)
    ins["g8"] = np.ones(512, np.float32)
    ins["be8"] = np.zeros(512, np.float32)
    ins["fc9_w"] = (rng.randn(10, 512) * 0.02).astype(np.float32)
    ins["fc9_b"] = np.zeros(10, np.float32)
    out = kernel(**ins)
    print("out", out.shape, out.dtype, np.abs(out).mean())
